# revision 1
# baseline (speedup 1.0000x reference)
"""Decoder block kernel for 8 Trainium2 NeuronCores.

Sharding: core = 2*b + h handles batch b, query tokens q with q % 2 == h
(interleaved so the causal-mask block structure is identical on every
core -> one SPMD program; the mask diagonal band differs only in DATA).

All activations live transposed [C, tokens] (C on partitions), so every
linear layer uses the stored [in,out] weights directly as the stationary
operand and no on-device transposes are needed. LayerNorm statistics are
computed with ones-matmuls on the PE (replicated across partitions);
softmax row sums come from a ones-column appended to V. Matmul operands
are float32r (full-rate fp32).
"""
import numpy as np

B, T, C, H, D, FF = 4, 1024, 1024, 16, 64, 4096
NT = C // 128   # 8 partition tiles of the model dim
KT = T // 128   # 8 context-token tiles
FT = FF // 128  # 32
TQ = T // 2     # 512 local query tokens per core

_CACHE = {}


def _build(repeat=1):
    import concourse.bacc as bacc
    import concourse.tile as tile
    from concourse import mybir

    nc = bacc.Bacc(None, target_bir_lowering=False)
    F32 = mybir.dt.float32
    F32R = mybir.dt.float32r

    def din(name, shape, dt=F32R):
        return nc.dram_tensor(name, shape, dt, kind="ExternalInput")

    t = {}
    t["xT"] = din("xT", [C, T])
    t["xTl"] = din("xTl", [C, TQ])
    t["encT"] = din("encT", [C, T])
    for k in ("wq1", "wk1", "wv1", "wo1", "wq2", "wk2", "wv2", "wo2"):
        t[k] = din(k, [C, C])
    t["wf1"] = din("wf1", [C, FF])        # pre-scaled by diag(g2)
    t["csq2"] = din("csq2", [C], F32)     # colsums of g1-scaled Wq2
    t["bq2"] = din("bq2", [C], F32)       # b1 @ Wq2
    t["csf1"] = din("csf1", [FF], F32)    # colsums of g2-scaled Wf1
    t["wf2"] = din("wf2", [FF, C])
    t["tri"] = din("tri", [128, 64])
    t["pad1"] = din("pad1", [T], F32)
    t["pad2"] = din("pad2", [T], F32)
    for k in ("g1", "b1", "g2", "b2", "g3", "b3"):
        t[k] = din(k, [C], F32)
    t["bf1"] = din("bf1", [FF], F32)
    t["bf2"] = din("bf2", [C], F32)
    t["outT"] = nc.dram_tensor("outT", [C, TQ], F32, kind="ExternalOutput")

    with tile.TileContext(nc) as tc:
        for it in range(repeat):
            _emit(nc, tc, t, it)
    nc.compile()
    return nc


def _emit(nc, tc, t, it):
    from contextlib import ExitStack
    import concourse.bass as bass
    from concourse import mybir
    from concourse.tile import add_dep_helper

    F32 = mybir.dt.float32
    F32R = mybir.dt.float32r
    AF = mybir.ActivationFunctionType
    ALU = mybir.AluOpType

    def vec_ap(dram, n):
        return bass.AP(tensor=dram, offset=0, ap=[[1, 128], [128, n // 128]])

    def w_ap(wdram, cout, ot, a0, na):
        """[128, na, 128] tile: W[128*(a0+a)+p, 128*ot+o]"""
        return bass.AP(tensor=wdram, offset=128 * ot + 128 * a0 * cout,
                       ap=[[cout, 128], [128 * cout, na], [1, 128]])

    with ExitStack() as ctx:
        consts = ctx.enter_context(tc.tile_pool(name=f"con{it}", bufs=1))
        ones128 = consts.tile([128, 128], F32R, tag="o128", name="o128")
        nc.vector.memset(ones128[:].bitcast(F32), 1.0)
        ones1 = consts.tile([1, 128], F32R, tag="o1", name="o1")
        nc.vector.memset(ones1[:].bitcast(F32), 1.0)
        eps_t = consts.tile([128, 1], F32, tag="eps", name="eps")
        nc.vector.memset(eps_t[:], 1e-5)
        tri_sb = consts.tile([128, 64], F32R, tag="tri", name="tri")

        def ldvec(dram, n, tagname):
            s = consts.tile([128, n // 128], F32, tag=tagname, name=tagname)
            nc.scalar.dma_start(s[:], vec_ap(dram, n))
            return s

        pad_sb = {"pad1": ldvec(t["pad1"], T, "pad1"),
                  "pad2": ldvec(t["pad2"], T, "pad2")}
        lv = {}

        def load_main_consts():
            lv.update(g1=ldvec(t["g1"], C, "g1"), b1=ldvec(t["b1"], C, "b1"),
                      g2=ldvec(t["g2"], C, "g2"), b2=ldvec(t["b2"], C, "b2"),
                      g3=ldvec(t["g3"], C, "g3"), b3=ldvec(t["b3"], C, "b3"),
                      bf1s=ldvec(t["bf1"], FF, "bf1"),
                      csq2s=ldvec(t["csq2"], C, "csq2"),
                      bq2s=ldvec(t["bq2"], C, "bq2"),
                      csf1s=ldvec(t["csf1"], FF, "csf1"),
                      bf2s=ldvec(t["bf2"], C, "bf2"))
            nc.scalar.dma_start(tri_sb[:], t["tri"][:])

        wpool = ctx.enter_context(tc.tile_pool(name=f"wp{it}", bufs=4))
        ypool = ctx.enter_context(tc.tile_pool(name=f"yp{it}", bufs=1))
        foldp = ctx.enter_context(tc.tile_pool(name=f"fp{it}", bufs=2))

        def fold_epilogue(ps, fold, ot, out_tile, func, bias_sb):
            """out = func(rstd*(ps - m*CS[ot]) + bias) given fold=(rstd, un)
            with un = -m*rstd, CS per-output-channel colsum."""
            rstd, un, cs = fold
            ftile = foldp.tile([128, TQ], F32, tag="ft", name="ft")
            nc.vector.tensor_mul(ftile[:], ps[:], rstd[:])
            nc.vector.scalar_tensor_tensor(
                out=ftile[:], in0=un[:], scalar=cs[:, ot:ot + 1], in1=ftile[:],
                op0=ALU.mult, op1=ALU.add)
            nc.scalar.activation(out_tile[:], ftile[:], func,
                                 bias=bias_sb[:, ot:ot + 1])

        def new_y(dt=F32R):
            return [ypool.tile([128, TQ], dt, tag=f"y{i}", name=f"y{i}")
                    for i in range(NT)]

        def linear_T(wdram, cin, cout, in_tiles, n, epilogue, pp):
            """psum[ot][:, q0:] = sum_ct W[ct,ot].T @ in[ct][:, q0:]"""
            nct = cin // 128
            for ot in range(cout // 128):
                wt = wpool.tile([128, nct, 128], F32R, tag="w", name="w")
                nc.sync.dma_start(wt[:], w_ap(wdram, cout, ot, 0, nct))
                for q0 in range(0, n, 512):
                    ps = pp.tile([128, 512], F32, tag="pp", name="pp")
                    for ct in range(nct):
                        nc.tensor.matmul(ps[:], wt[:, ct, :],
                                         in_tiles[ct][:, q0:q0 + 512],
                                         start=(ct == 0), stop=(ct == nct - 1))
                    epilogue(ot, q0, ps)

        def ln_begin(sctx, tagp, lnps, lntag):
            sqp = sctx.enter_context(tc.tile_pool(name=f"sq{tagp}{it}", bufs=2))
            scr = sctx.enter_context(tc.tile_pool(name=f"ls{tagp}{it}", bufs=1))
            ps1 = lnps.tile([128, TQ], F32, tag=lntag, name="ln")
            ps2 = lnps.tile([128, TQ], F32, tag=lntag, name="ln")
            return {"sqp": sqp, "scr": scr, "ps1": ps1, "ps2": ps2}

        def ln_feed(st, y_tile, ct):
            """Accumulate sum(y) and sum(y^2) for one partition tile."""
            sq = st["sqp"].tile([128, TQ], F32R, tag="sq", name="sq")
            with nc.allow_low_precision(reason="fp32r rounding ok"):
                nc.vector.tensor_mul(sq[:], y_tile[:], y_tile[:])
            nc.tensor.matmul(st["ps1"][:], ones128[:], y_tile[:],
                             start=(ct == 0), stop=(ct == NT - 1),
                             skip_group_check=True)
            nc.tensor.matmul(st["ps2"][:], ones128[:], sq[:],
                             start=(ct == 0), stop=(ct == NT - 1),
                             skip_group_check=True)

        def ln_finish(st, y_in, g, b, out_t, statpool=None):
            """Scalar chain + normalize (overwrites y_in) + affine.
            If statpool is given, m/rstd/un are allocated there and
            (rstd, un) returned for fold_epilogue use by the next stage."""
            scr = st["scr"]
            sp_ = statpool if statpool is not None else scr
            m = sp_.tile([128, TQ], F32, tag="m", name="m")
            nc.vector.tensor_scalar_mul(m[:], st["ps1"][:], 1.0 / C)
            ms = scr.tile([128, TQ], F32, tag="v", name="v")
            nc.vector.tensor_scalar_mul(ms[:], st["ps2"][:], 1.0 / C)
            m2 = scr.tile([128, TQ], F32, tag="v2", name="v2")
            nc.vector.tensor_mul(m2[:], m[:], m[:])
            nc.vector.tensor_sub(ms[:], ms[:], m2[:])
            nc.scalar.activation(ms[:], ms[:], AF.Sqrt, bias=eps_t[:])
            rstd = sp_.tile([128, TQ], F32, tag="r", name="r")
            nc.vector.reciprocal(rstd[:], ms[:])
            un = None
            if statpool is not None:
                un = sp_.tile([128, TQ], F32, tag="un", name="un")
                nc.vector.tensor_mul(un[:], m[:], rstd[:])
                nc.vector.tensor_scalar(un[:], un[:], -1.0, None,
                                        op0=ALU.mult)
            for ct in range(NT):
                eng = nc.vector if ct % 2 == 0 else nc.gpsimd
                d = scr.tile([128, TQ], F32, tag="d", name="d", bufs=2)
                eng.tensor_sub(d[:], y_in[ct][:], m[:])
                eng.tensor_mul(d[:], d[:], rstd[:])
                nc.scalar.activation(out_t[ct][:], d[:], AF.Identity,
                                     bias=b[:, ct:ct + 1], scale=g[:, ct:ct + 1])
            return (rstd, un)

        def ln_finish_chunked(st, y_in, g, b, out_t, nch=2):
            """Tail-optimized finish: process column chunks so the output
            DMAs of early chunks overlap the normalize of later ones."""
            scr = st["scr"]
            m = scr.tile([128, TQ], F32, tag="m", name="m")
            ms = scr.tile([128, TQ], F32, tag="v", name="v")
            m2 = scr.tile([128, TQ], F32, tag="v2", name="v2")
            rstd = scr.tile([128, TQ], F32, tag="r", name="r")
            w = TQ // nch
            for cch in range(nch):
                sl = slice(cch * w, (cch + 1) * w)
                nc.vector.tensor_scalar_mul(m[:, sl], st["ps1"][:, sl], 1.0 / C)
                nc.vector.tensor_scalar_mul(ms[:, sl], st["ps2"][:, sl],
                                            1.0 / C)
                nc.vector.tensor_mul(m2[:, sl], m[:, sl], m[:, sl])
                nc.vector.tensor_sub(ms[:, sl], ms[:, sl], m2[:, sl])
                nc.scalar.activation(ms[:, sl], ms[:, sl], AF.Sqrt,
                                     bias=eps_t[:])
                nc.vector.reciprocal(rstd[:, sl], ms[:, sl])
                for ct in range(NT):
                    eng = nc.vector if ct % 2 == 0 else nc.gpsimd
                    d = scr.tile([128, w], F32, tag="dc", name="dc", bufs=3)
                    eng.tensor_sub(d[:], y_in[ct][:, sl], m[:, sl])
                    eng.tensor_mul(d[:], d[:], rstd[:, sl])
                    nc.scalar.activation(out_t[ct][:, sl], d[:], AF.Identity,
                                         bias=b[:, ct:ct + 1],
                                         scale=g[:, ct:ct + 1])

        def kv_stage(src_dram, wk_d, wv_d, wq_d, q_src, padkey, kTt, vtt, qTt,
                     sctx, tagp, q_first, pp, qfold=None, src_after=None):
            """Load transposed source; compute K_T, V (pad-masked, with the
            pad column in slot 64 for the softmax row sums), and Q_T.
            q_first=True when the Q source is already resident (emit Q's
            matmuls before the big src DMAs); False when Q's source is the
            previous stage's LN output (emit K/V first so the PE can run
            while the LN chain finishes)."""
            with ExitStack() as kctx:
                sp = kctx.enter_context(tc.tile_pool(name=f"sr{tagp}{it}",
                                                     bufs=1))

                def emit_q():
                    if qfold is None:
                        linear_T(wq_d, C, C, q_src, TQ,
                                 lambda ot, q0, ps: nc.scalar.copy(
                                     qTt[ot][:], ps[:]), pp)
                    else:
                        linear_T(wq_d, C, C, q_src, TQ,
                                 lambda ot, q0, ps: fold_epilogue(
                                     ps, qfold, ot, qTt[ot], AF.Identity,
                                     lv["bq2s"]), pp)

                src_t = sp.tile([128, NT, T], F32R, tag="s", name="s")
                src = [src_t[:, i, :] for i in range(NT)]
                if q_first:
                    emit_q()
                src_eng = nc.gpsimd if q_first else nc.sync
                for th in range(2):
                    di = src_eng.dma_start(
                        src_t[:, :, 512 * th:512 * (th + 1)],
                        bass.AP(tensor=src_dram, offset=512 * th,
                                ap=[[T, 128], [128 * T, NT], [1, 512]]))
                    if th == 0 and src_after is not None:
                        add_dep_helper(di.ins, src_after.ins,
                                       reason="xT transfer after xTl")
                linear_T(wk_d, C, C, src, T,
                         lambda ot, q0, ps: nc.scalar.copy(
                             kTt[ot][:, q0:q0 + 512], ps[:]), pp)
                if not q_first:
                    emit_q()
                wvp = kctx.enter_context(tc.tile_pool(name=f"wv{tagp}{it}",
                                                      bufs=2))
                for tt in range(KT):
                    nc.scalar.dma_start(
                        vtt[tt][:, :, 64:65],
                        bass.AP(tensor=t[padkey], offset=128 * tt,
                                ap=[[1, 128], [0, 16], [0, 1]]).bitcast(F32R))
                for half in range(2):
                    wvq = []
                    for cq in range(4):
                        wvt = wvp.tile([128, 2, 512], F32R, tag="wv",
                                       name="wv", bufs=4)
                        nc.sync.dma_start(
                            wvt[:],
                            bass.AP(tensor=wv_d,
                                    offset=512 * half + 256 * cq * C,
                                    ap=[[C, 128], [128 * C, 2], [1, 512]]))
                        wvq.append(wvt)
                    for tt in range(KT):
                        ps = pp.tile([128, 512], F32, tag="pp", name="pp")
                        for ct in range(NT):
                            nc.tensor.matmul(
                                ps[:], src[ct][:, 128 * tt:128 * (tt + 1)],
                                wvq[ct // 2][:, ct % 2, :],
                                start=(ct == 0), stop=(ct == NT - 1))
                        nc.vector.tensor_scalar_mul(
                            vtt[tt][:, 8 * half:8 * (half + 1), 0:64],
                            ps[:].rearrange("p (h d) -> p h d", d=64),
                            pad_sb[padkey][:, tt:tt + 1])

        def attention(qTt, kTt, vtt, wo_d, resid, g, b, out_t, causal,
                      sctx, tagp, scp, avp, rbp, op, statpool=None):
            """Scores/AV in transposed layout; O-projection output (+resid)
            is written back into the qT tiles (dead by then), then LN."""
            with ExitStack() as atx:
                pvp = atx.enter_context(tc.tile_pool(
                    name=f"pv{tagp}{it}", bufs=1))
                ppool = atx.enter_context(tc.tile_pool(
                    name=f"pt{tagp}{it}", bufs=4))
                sbp = atx.enter_context(tc.tile_pool(
                    name=f"sb{tagp}{it}", bufs=2))
                pv = [pvp.tile([128, TQ], F32R, tag=f"pv{i}", name=f"pv{i}")
                      for i in range(NT)]
                for h in range(H):
                    ct, off = h // 2, (h % 2) * 64
                    av = avp.tile([65, 512], F32, tag="av", name="av")
                    pend = None

                    def emit_av(p, c, k):
                        nc.tensor.matmul(av[:, c:512], vtt[k][:, h, 0:65],
                                         p[:, c:512],
                                         start=(k == 0), stop=(k == KT - 1))

                    for kt in range(KT):
                        c0 = min(64 * kt, 256) if causal else 0
                        sp = scp.tile([128, 512], F32, tag="pp", name="pp")
                        Pt = ppool.tile([128, TQ], F32R, tag="P", name="P")
                        nc.tensor.matmul(
                            sp[:, c0:512],
                            kTt[ct][off:off + 64, 128 * kt:128 * (kt + 1)],
                            qTt[ct][off:off + 64, c0:512],
                            start=True, stop=True)
                        nc.scalar.activation(Pt[:, c0:512], sp[:, c0:512],
                                             AF.Exp, scale=0.125)
                        if causal:
                            if 64 * kt > 256:
                                nc.gpsimd.memset(Pt[:, 256:64 * kt].bitcast(F32), 0.0)
                            nc.gpsimd.tensor_mul(
                                Pt[:, 64 * kt:64 * (kt + 1)],
                                Pt[:, 64 * kt:64 * (kt + 1)], tri_sb[:])
                        if pend is not None:
                            emit_av(*pend)
                        pend = (Pt, c0, kt)
                    emit_av(*pend)
                    rinv = sbp.tile([1, 512], F32R, tag="ri", name="ri")
                    with nc.allow_low_precision(reason="fp32r rounding ok"):
                        nc.vector.reciprocal(rinv[:], av[64:65, :])
                    rb_ps = rbp.tile([128, 512], F32, tag="rb", name="rb")
                    nc.tensor.matmul(rb_ps[:], ones1[:], rinv[:],
                                     start=True, stop=True)
                    rb = sbp.tile([64, 512], F32, tag="rs", name="rs")
                    nc.vector.tensor_copy(rb[:], rb_ps[0:64, :])
                    with nc.allow_low_precision(reason="fp32r rounding ok"):
                        nc.vector.tensor_mul(pv[ct][off:off + 64, :],
                                             av[0:64, :], rb[:])
                lnst = ln_begin(atx, tagp, avp, "av")
                for co in range(NT):
                    wt = wpool.tile([128, NT, 128], F32R, tag="w", name="w")
                    nc.sync.dma_start(wt[:], w_ap(wo_d, C, co, 0, NT))
                    ps = op.tile([128, 512], F32, tag="o", name="o")
                    for cc in range(NT):
                        nc.tensor.matmul(ps[:], wt[:, cc, :], pv[cc][:],
                                         start=(cc == 0), stop=(cc == NT - 1))
                    with nc.allow_low_precision(reason="fp32r rounding ok"):
                        nc.vector.tensor_add(qTt[co][:], ps[:], resid[co][:])
                    ln_feed(lnst, qTt[co], co)
                return ln_finish(lnst, qTt, g, b, out_t, statpool)

        # ================= main flow =================
        qpool = ctx.enter_context(tc.tile_pool(name=f"qp{it}", bufs=1))
        statp = ctx.enter_context(tc.tile_pool(name=f"st{it}", bufs=1))

        def new_q(pfx="q"):
            return [qpool.tile([128, TQ], F32R, tag=f"{pfx}{i}",
                               name=f"{pfx}{i}") for i in range(NT)]

        with ExitStack() as actx:
            qkv = actx.enter_context(tc.tile_pool(name=f"qkv{it}", bufs=1))
            pps = actx.enter_context(tc.tile_pool(
                name=f"ps{it}", bufs=3, space="PSUM"))
            avps = actx.enter_context(tc.tile_pool(
                name=f"as{it}", bufs=2, space="PSUM"))
            rbps = actx.enter_context(tc.tile_pool(
                name=f"rs{it}", bufs=1, space="PSUM"))
            ops = actx.enter_context(tc.tile_pool(
                name=f"os{it}", bufs=2, space="PSUM"))

            def new_kv():
                k = [qkv.tile([128, T], F32R, tag=f"k{i}", name=f"k{i}")
                     for i in range(NT)]
                v = [qkv.tile([128, 16, 65], F32R, tag=f"v{i}", name=f"v{i}")
                     for i in range(KT)]
                return k, v

            # ---- self-attention + AddNorm ----
            qT = new_q()
            kTt, vtt = new_kv()
            with ExitStack() as sctx:
                xTl_sb = new_q("x")
                last_xtl = None
                for i in range(NT):
                    last_xtl = nc.sync.dma_start(
                        xTl_sb[i][:], t["xTl"][128 * i:128 * (i + 1), :])
                kv_stage(t["xT"], t["wk1"], t["wv1"], t["wq1"], xTl_sb,
                         "pad1", kTt, vtt, qT, sctx, "s", True, pps,
                         src_after=last_xtl)
                load_main_consts()
                y1 = new_y()
                fold1 = attention(qT, kTt, vtt, t["wo1"], xTl_sb,
                                  lv["g1"], lv["b1"], y1, True, sctx, "s",
                                  pps, avps, rbps, ops, statpool=statp)

            # ---- cross-attention + AddNorm (fresh tile generations) ----
            qT2 = new_q("x")
            kTt2, vtt2 = new_kv()
            with ExitStack() as cctx:
                kv_stage(t["encT"], t["wk2"], t["wv2"], t["wq2"], qT,
                         "pad2", kTt2, vtt2, qT2, cctx, "c", False, pps,
                         qfold=(fold1[0], fold1[1], lv["csq2s"]))
                y2 = new_y()
                fold2 = attention(qT2, kTt2, vtt2, t["wo2"], y1,
                                  lv["g2"], lv["b2"], y2, False, cctx, "c",
                                  pps, avps, rbps, ops, statpool=statp)

        # ---- FFN + AddNorm ----
        with ExitStack() as fctx:
            y3p = fctx.enter_context(tc.tile_pool(name=f"y3{it}", bufs=1))
            y3 = [y3p.tile([128, TQ], F32, tag=f"z{i}", name=f"z{i}")
                  for i in range(NT)]
            lnps3 = fctx.enter_context(tc.tile_pool(
                name=f"l3{it}", bufs=2, space="PSUM"))
            lnst3 = ln_begin(fctx, "f", lnps3, "ln")
            ffold = (fold2[0], fold2[1], lv["csf1s"])
            with ExitStack() as mctx:
                hp = mctx.enter_context(tc.tile_pool(name=f"hp{it}", bufs=1))
                w1p = mctx.enter_context(tc.tile_pool(name=f"w1{it}", bufs=4))
                w2p = mctx.enter_context(tc.tile_pool(name=f"w2{it}", bufs=2))
                pp1 = mctx.enter_context(tc.tile_pool(
                    name=f"f1{it}", bufs=3, space="PSUM"))
                pp2 = mctx.enter_context(tc.tile_pool(
                    name=f"f2{it}", bufs=2, space="PSUM"))
                NF = 16
                for fb in range(2):
                    h_sb = [hp.tile([128, TQ], F32R, tag=f"h{i}",
                                    name=f"h{i}") for i in range(NF)]
                    for f in range(NF):
                        fg = fb * NF + f
                        w1t = w1p.tile([128, NT, 128], F32R, tag="w1",
                                       name="w1")
                        nc.sync.dma_start(w1t[:], w_ap(t["wf1"], FF, fg, 0, NT))
                        ps = pp1.tile([128, 512], F32, tag="p1", name="p1")
                        for ct in range(NT):
                            nc.tensor.matmul(ps[:], w1t[:, ct, :], qT2[ct][:],
                                             start=(ct == 0),
                                             stop=(ct == NT - 1))
                        fold_epilogue(ps, ffold, fg, h_sb[f], AF.Relu, lv["bf1s"])
                    for co in range(NT):
                        w2t = w2p.tile([128, NF, 128], F32R, tag="w2",
                                       name="w2")
                        nc.sync.dma_start(
                            w2t[:], w_ap(t["wf2"], C, co, fb * NF, NF))
                        ps = pp2.tile([128, 512], F32, tag="p2", name="p2")
                        for f in range(NF):
                            nc.tensor.matmul(ps[:], w2t[:, f, :], h_sb[f][:],
                                             start=(f == 0),
                                             stop=(f == NF - 1))
                        if fb == 0:
                            nc.vector.tensor_copy(y3[co][:], ps[:])
                        else:
                            nc.vector.scalar_tensor_tensor(
                                out=y3[co][:], in0=ps[:],
                                scalar=lv["bf2s"][:, co:co + 1], in1=y3[co][:],
                                op0=ALU.add, op1=ALU.add)
                            with nc.allow_low_precision(
                                    reason="fp32r rounding ok"):
                                nc.vector.tensor_add(y2[co][:], y3[co][:],
                                                     y2[co][:])
                            ln_feed(lnst3, y2[co], co)
            ln_finish_chunked(lnst3, y2, lv["g3"], lv["b3"], y3)
            for cch in range(2):
                sl = slice(cch * (TQ // 2), (cch + 1) * (TQ // 2))
                for co in range(NT):
                    nc.sync.dma_start(
                        t["outT"][128 * co:128 * (co + 1), sl],
                        y3[co][:, sl])


def _shard(inputs):
    x = np.ascontiguousarray(np.asarray(inputs["x"], dtype=np.float32))
    enc = np.ascontiguousarray(np.asarray(inputs["enc_out"], dtype=np.float32))
    tpad = np.asarray(inputs["tgt_pad_mask"]).astype(np.float32)
    spad = np.asarray(inputs["src_pad_mask"]).astype(np.float32)
    ws = {k: np.ascontiguousarray(np.asarray(inputs[k], dtype=np.float32))
          for k in ("Wq1", "Wk1", "Wv1", "Wo1", "Wq2", "Wk2", "Wv2", "Wo2",
                    "Wf1", "Wf2")}
    lnv = {k: np.ascontiguousarray(np.asarray(inputs[k], dtype=np.float32))
           for k in ("ln1_g", "ln1_b", "ln2_g", "ln2_b", "ln3_g", "ln3_b",
                     "bf1", "bf2")}
    # LN1 affine folded through Wq2; LN2 affine folded through Wf1.
    wq2f = np.ascontiguousarray(lnv["ln1_g"][:, None] * ws["Wq2"])
    csq2 = np.ascontiguousarray(wq2f.sum(axis=0))
    bq2 = np.ascontiguousarray(lnv["ln1_b"] @ ws["Wq2"])
    wf1f = np.ascontiguousarray(lnv["ln2_g"][:, None] * ws["Wf1"])
    csf1 = np.ascontiguousarray(wf1f.sum(axis=0))
    bf1f = np.ascontiguousarray(lnv["bf1"] + lnv["ln2_b"] @ ws["Wf1"])
    in_maps = []
    for b in range(B):
        xTb = np.ascontiguousarray(x[b].T)
        eTb = np.ascontiguousarray(enc[b].T)
        p1 = np.ascontiguousarray(1.0 - tpad[b])
        p2 = np.ascontiguousarray(1.0 - spad[b])
        for h in range(2):
            xTlb = np.ascontiguousarray(x[b, h::2, :].T)
            trih = (np.arange(128)[:, None] <= 2 * np.arange(64)[None, :] + h
                    ).astype(np.float32)
            in_maps.append({
                "xT": xTb, "xTl": xTlb, "encT": eTb,
                "wq1": ws["Wq1"], "wk1": ws["Wk1"], "wv1": ws["Wv1"],
                "wo1": ws["Wo1"], "wq2": wq2f, "wk2": ws["Wk2"],
                "wv2": ws["Wv2"], "wo2": ws["Wo2"],
                "wf1": wf1f, "wf2": ws["Wf2"],
                "csq2": csq2, "bq2": bq2, "csf1": csf1,
                "tri": np.ascontiguousarray(trih),
                "pad1": p1, "pad2": p2,
                "g1": lnv["ln1_g"], "b1": lnv["ln1_b"],
                "g2": lnv["ln2_g"], "b2": lnv["ln2_b"],
                "g3": lnv["ln3_g"], "b3": lnv["ln3_b"],
                "bf1": bf1f, "bf2": lnv["bf2"],
            })
    return in_maps


def _get_nc(repeat=1):
    if repeat not in _CACHE:
        _CACHE[repeat] = _build(repeat)
    return _CACHE[repeat]


def kernel(**inputs):
    from concourse.bass_utils import run_bass_kernel_spmd
    nc = _get_nc()
    in_maps = _shard(inputs)
    res = run_bass_kernel_spmd(nc, in_maps, core_ids=list(range(8)))
    out = np.empty((B, T, C), np.float32)
    for core in range(8):
        b, h = core // 2, core % 2
        out[b, h::2, :] = res.results[core]["outT"].T
    return out



# revision 14
# speedup vs baseline: 1.2340x; 1.2340x over previous
"""Decoder block kernel for 8 Trainium2 NeuronCores — fp8/bf16 version.

Sharding: core = 2*b + h handles batch b, query tokens q with q % 2 == h
(interleaved so the causal-mask structure is identical on every core).

Precision plan (validated against the fp32 reference in numpy, max rel
err ~1.2e-2 vs the 2e-2 gate):
- Q/K/V projections: fp8e4m3 DoubleRow (both operands fp8, contraction
  pairs packed), 4x the bf16 matmul rate.  Activations and weights are
  pre-scaled by powers of two so fp8's 3-bit mantissa covers them; the
  scales fold into existing epilogue constants.
- Attention scores: bf16 (the d=64 contraction can't pack into
  DoubleRow partitions without a transpose).  True causal lower bound
  per key tile (bf16 runs full rate at any free size).
- P = exp(score/8)*SP written fp8 by the activation op (bias=ln SP);
  AV in fp8 DoubleRow over key-tile pairs; the pad column of V doubles
  as the softmax denominator accumulator.
- O projections: bf16 (error budget).
- FFN1: fp8 DoubleRow with both operands 2-term same-scale splits
  (W ~ (A+B)/SW, x2 split on device) -> 3 accumulation passes into one
  psum group.  FFN2: weights split only (h single-fp8).
- LayerNorm statistics via ones-matmuls on bf16; LN1/LN2 affines folded
  through Wq2/Wf1 (colsums of the *quantized* weights for exact mean
  cancellation).
"""
import numpy as np

B, T, C, H, D, FF = 4, 1024, 1024, 16, 64, 4096
NT = C // 128   # 8 partition tiles of the model dim
KT = T // 128   # 8 context-token tiles
FT = FF // 128  # 32
TQ = T // 2     # 512 local query tokens per core

SW = 1024.0     # weight fp8 scale
SX = 16.0       # x / enc fp8 scale
SV = 32.0       # V fp8 scale
SP = 8.0        # P fp8 scale
SY1 = 16.0      # y1 (pre-LN1) fp8 scale
SY2 = 16.0      # y2 (pre-LN2) fp8 scale
SH = 32.0       # h (FFN hidden) fp8 scale

_CACHE = {}


def _build(repeat=1):
    import concourse.bacc as bacc
    import concourse.tile as tile
    from concourse import mybir

    nc = bacc.Bacc(None, target_bir_lowering=False)
    F32 = mybir.dt.float32
    F8 = mybir.dt.float8e4
    BF = mybir.dt.bfloat16

    t = {}

    def din(name, shape, dt=F8):
        t[name] = nc.dram_tensor(name, shape, dt, kind="ExternalInput")

    din("xT8", [C, T])
    din("xl8", [C, TQ])
    din("encT8", [C, T])
    din("xl_bf", [C, TQ], BF)
    # stationary weights pre-tiled [n_out_blocks, 128, cin]
    for k in ("wq18", "wk18", "wq28", "wk28"):
        din(k, [NT, 128, C])
    # V weights raw [cin, cout] (moving operand)
    din("wv18", [C, C])
    din("wv28", [C, C])
    din("wo1bf", [NT, 128, C], BF)
    din("wo2bf", [NT, 128, C], BF)
    din("wf1ab", [FT, 128, 2 * C])          # [fg][p][ab*C + k]
    din("wf2ab", [NT, 128, 2 * 2 * 2048])   # [co][p][ab][fb][2048]
    din("tri8", [128, 64])
    din("pad18", [T])
    din("pad28", [T])
    din("csq2", [C], F32)
    din("bq2", [C], F32)
    din("pad1s", [T], F32)
    din("pad2s", [T], F32)
    din("csf1", [FF], F32)
    din("bf1s", [FF], F32)
    din("bf2s", [C], F32)
    for k in ("g1", "b1", "g2", "b2", "g3", "b3"):
        din(k, [C], F32)
    t["outT"] = nc.dram_tensor("outT", [C, TQ], F32, kind="ExternalOutput")

    with tile.TileContext(nc) as tc:
        for it in range(repeat):
            _emit(nc, tc, t, it)
    nc.compile()
    return nc


def _emit(nc, tc, t, it):
    from contextlib import ExitStack
    import math
    import concourse.bass as bass
    from concourse import mybir

    F32 = mybir.dt.float32
    F32R = mybir.dt.float32r
    F8 = mybir.dt.float8e4
    BF = mybir.dt.bfloat16
    AF = mybir.ActivationFunctionType
    ALU = mybir.AluOpType
    DR = mybir.MatmulPerfMode.DoubleRow

    cQK = 1.0 / (SX * SW)        # Q/K psum -> bf16
    cQ2 = 1.0 / (SY1 * SW)       # Q2 fold rstd scale
    cF1 = 1.0 / (SY2 * SW)       # FFN1 fold rstd scale
    cF2 = 1.0 / (SH * SW)        # FFN2 psum -> value

    def vec_ap(dram, n):
        return bass.AP(tensor=dram, offset=0, ap=[[1, 128], [128, n // 128]])

    def wq_ap(wdram, ot):
        """Pre-tiled stationary block ot: [128, NT, 128]."""
        return bass.AP(tensor=wdram, offset=ot * 128 * C,
                       ap=[[C, 128], [128, NT], [1, 128]])

    def chunk_ap(dram, n, jp):
        """ct-pair chunk of a [C, n] tensor: [128, 2, n]."""
        return bass.AP(tensor=dram, offset=256 * jp * n,
                       ap=[[n, 128], [128 * n, 2], [1, n]])

    with ExitStack() as ctx:
        consts = ctx.enter_context(tc.tile_pool(name=f"con{it}", bufs=1))
        ones128 = consts.tile([128, 128], BF, tag="o128", name="o128")
        nc.vector.memset(ones128[:], 1.0)
        ones128r = consts.tile([128, 128], F32R, tag="o128r", name="o128r")
        nc.vector.memset(ones128r[:].bitcast(F32), 1.0)
        ones1 = consts.tile([1, 128], F32R, tag="o1", name="o1")
        nc.vector.memset(ones1[:].bitcast(F32), 1.0 / SV)
        eps_t = consts.tile([128, 1], F32, tag="eps", name="eps")
        nc.vector.memset(eps_t[:], 1e-5)
        expb = consts.tile([128, 1], F32, tag="expb", name="expb")
        nc.vector.memset(expb[:], math.log(SP))
        tri_sb = consts.tile([128, 64], F8, tag="tri", name="tri")

        def ldvec(key, n):
            s = consts.tile([128, n // 128], F32, tag=key, name=key)
            nc.scalar.dma_start(s[:], vec_ap(t[key], n))
            return s

        lv = {}

        def load_main_consts():
            for key, n in (("g1", C), ("b1", C), ("g2", C), ("b2", C),
                           ("g3", C), ("b3", C), ("csq2", C), ("bq2", C),
                           ("csf1", FF), ("bf1s", FF), ("bf2s", C)):
                lv[key] = ldvec(key, n)
            nc.scalar.dma_start(tri_sb[:], t["tri8"][:])

        pad_sb = {"pad1s": ldvec("pad1s", T), "pad2s": ldvec("pad2s", T)}

        wpool = ctx.enter_context(tc.tile_pool(name=f"wp{it}", bufs=4))
        foldp = ctx.enter_context(tc.tile_pool(name=f"fp{it}", bufs=2))
        statp = ctx.enter_context(tc.tile_pool(name=f"st{it}", bufs=1))

        # ---------- LN helpers ----------
        def ln_begin(sctx, tagp, lnps=None, lntag=None):
            sqp = sctx.enter_context(tc.tile_pool(name=f"sq{tagp}{it}", bufs=2))
            scr = sctx.enter_context(tc.tile_pool(name=f"ls{tagp}{it}", bufs=1))
            st = {"sqp": sqp, "scr": scr, "ps1": None, "ps2": None}
            if lnps is not None:
                ln_start_stats(st, lnps, lntag)
            return st

        def ln_start_stats(st, lnps, lntag):
            st["ps1"] = lnps.tile([128, TQ], F32, tag=lntag, name="ln")
            st["ps2"] = lnps.tile([128, TQ], F32, tag=lntag, name="ln")

        def ln_feed(st, y_tile, ct, f32=False):
            sq = st["sqp"].tile([128, TQ], BF, tag="sq", name="sq")
            with nc.allow_low_precision(reason="bf16 stats ok"):
                nc.vector.tensor_mul(sq[:], y_tile[:], y_tile[:])
            y_ap = y_tile[:].bitcast(F32R) if f32 else y_tile[:]
            ones = ones128r if f32 else ones128
            nc.tensor.matmul(st["ps1"][:], ones[:], y_ap,
                             start=(ct == 0), stop=(ct == NT - 1),
                             skip_group_check=True)
            nc.tensor.matmul(st["ps2"][:], ones128[:], sq[:],
                             start=(ct == 0), stop=(ct == NT - 1),
                             skip_group_check=True)

        def ln_finish(st, y_in, g, b, out_t, fold_c=None):
            """Stats; normalized+affine out_t (bf16).  With fold_c also
            produce (r2, un): r2 = rstd*fold_c, un = -m*rstd/SW."""
            scr = st["scr"]
            m = scr.tile([128, TQ], F32, tag="m", name="m")
            nc.vector.tensor_scalar_mul(m[:], st["ps1"][:], 1.0 / C)
            ms = scr.tile([128, TQ], F32, tag="v", name="v")
            nc.vector.tensor_scalar_mul(ms[:], st["ps2"][:], 1.0 / C)
            m2 = scr.tile([128, TQ], F32, tag="v2", name="v2")
            nc.vector.tensor_mul(m2[:], m[:], m[:])
            nc.vector.tensor_sub(ms[:], ms[:], m2[:])
            nc.scalar.activation(ms[:], ms[:], AF.Sqrt, bias=eps_t[:])
            rstd = scr.tile([128, TQ], F32, tag="r", name="r")
            nc.vector.reciprocal(rstd[:], ms[:])
            ret = None
            if fold_c is not None:
                un = statp.tile([128, TQ], F32, tag="un", name="un")
                nc.vector.tensor_mul(un[:], m[:], rstd[:])
                nc.vector.tensor_scalar(un[:], un[:], -1.0 / SW, None,
                                        op0=ALU.mult)
                r2 = statp.tile([128, TQ], F32, tag="r2", name="r2")
                nc.vector.tensor_scalar_mul(r2[:], rstd[:], fold_c)
                ret = (r2, un)
            for ct in range(NT):
                eng = nc.vector if ct % 2 == 0 else nc.gpsimd
                d = scr.tile([128, TQ], F32, tag="d", name="d", bufs=2)
                eng.tensor_sub(d[:], y_in[ct][:], m[:])
                eng.tensor_mul(d[:], d[:], rstd[:])
                with nc.allow_low_precision(reason="bf16 ln out"):
                    eng.tensor_scalar(out_t[ct][:], d[:], g[:, ct:ct + 1],
                                      b[:, ct:ct + 1], op0=ALU.mult,
                                      op1=ALU.add)
            return ret

        def ln_finish_chunked(st, y_in, g, b, out_t, nch=4):
            scr = st["scr"]
            m = scr.tile([128, TQ], F32, tag="m", name="m")
            ms = scr.tile([128, TQ], F32, tag="v", name="v")
            m2 = scr.tile([128, TQ], F32, tag="v2", name="v2")
            rstd = scr.tile([128, TQ], F32, tag="r", name="r")
            w = TQ // nch
            for cch in range(nch):
                sl = slice(cch * w, (cch + 1) * w)
                nc.vector.tensor_scalar_mul(m[:, sl], st["ps1"][:, sl], 1.0 / C)
                nc.vector.tensor_scalar_mul(ms[:, sl], st["ps2"][:, sl], 1.0 / C)
                nc.vector.tensor_mul(m2[:, sl], m[:, sl], m[:, sl])
                nc.vector.tensor_sub(ms[:, sl], ms[:, sl], m2[:, sl])
                nc.scalar.activation(ms[:, sl], ms[:, sl], AF.Sqrt,
                                     bias=eps_t[:])
                nc.vector.reciprocal(rstd[:, sl], ms[:, sl])
                for ct in range(NT):
                    eng = nc.vector if ct % 2 == 0 else nc.gpsimd
                    d = scr.tile([128, w], F32, tag="dc", name="dc", bufs=3)
                    eng.tensor_sub(d[:], y_in[ct][:, sl], m[:, sl])
                    eng.tensor_mul(d[:], d[:], rstd[:, sl])
                    eng.tensor_scalar(out_t[ct][:, sl], d[:], g[:, ct:ct + 1],
                                      b[:, ct:ct + 1], op0=ALU.mult,
                                      op1=ALU.add)

        def fold_epilogue(ps, fold, cs, ot, out_ap, func, bias_sb, scale):
            """out = func(scale*(r2*ps + un*cs[ot]) + bias[ot])"""
            r2, un = fold
            ftile = foldp.tile([128, TQ], F32, tag="ft", name="ft")
            nc.vector.tensor_mul(ftile[:], ps[:], r2[:])
            nc.vector.scalar_tensor_tensor(
                out=ftile[:], in0=un[:], scalar=cs[:, ot:ot + 1], in1=ftile[:],
                op0=ALU.mult, op1=ALU.add)
            nc.scalar.activation(out_ap, ftile[:], func,
                                 bias=bias_sb[:, ot:ot + 1], scale=scale)

        # ---------- projection helpers ----------
        def qkv_proj(src8, w8_key, n, out_fn, pp):
            for ot in range(NT):
                wt = wpool.tile([128, NT, 128], F8, tag="w", name="w")
                nc.sync.dma_start(wt[:], wq_ap(t[w8_key], ot))
                for q0 in range(0, n, 512):
                    ps = pp.tile([128, 512], F32, tag="pp", name="pp")
                    for jp in range(4):
                        nc.tensor.matmul(
                            ps[:], wt[:, 2 * jp:2 * jp + 2, :],
                            src8[:, 2 * jp:2 * jp + 2, q0:q0 + 512],
                            start=(jp == 0), stop=(jp == 3), perf_mode=DR)
                    out_fn(ot, q0, ps)

        def v_proj(src8, wv_key, vtt, pad8key, padskey, pp):
            wvq = []
            for half in range(2):
                wvt = wpool.tile([128, NT, 512], F8, tag="wv", name="wv",
                                 bufs=2)
                nc.sync.dma_start(
                    wvt[:], bass.AP(tensor=t[wv_key], offset=512 * half,
                                    ap=[[C, 128], [128 * C, NT], [1, 512]]))
                wvq.append(wvt)
            for p in range(4):
                for s in range(2):
                    tt = 2 * p + s
                    nc.scalar.dma_start(
                        vtt[p][:, s, :, 64:65],
                        bass.AP(tensor=t[pad8key], offset=128 * tt,
                                ap=[[1, 128], [0, H], [0, 1]]))
                    for half in range(2):
                        ps = pp.tile([128, 512], F32, tag="pp", name="pp")
                        for jp in range(4):
                            nc.tensor.matmul(
                                ps[:],
                                src8[:, 2 * jp:2 * jp + 2,
                                     128 * tt:128 * (tt + 1)],
                                wvq[half][:, 2 * jp:2 * jp + 2, :],
                                start=(jp == 0), stop=(jp == 3), perf_mode=DR)
                        with nc.allow_low_precision(reason="fp8 V"):
                            nc.vector.tensor_scalar_mul(
                                vtt[p][:, s, 8 * half:8 * (half + 1), 0:64],
                                ps[:].rearrange("p (h d) -> p h d", d=64),
                                pad_sb[padskey][:, tt:tt + 1])

        def attention(qTt, kTt, vtt, wo_key, resid, out_y, causal,
                      pps, avps, rbps, ops, lnst):
            with ExitStack() as atx:
                pvp = atx.enter_context(tc.tile_pool(name=f"pv{wo_key}{it}",
                                                     bufs=1))
                ppool = atx.enter_context(tc.tile_pool(name=f"pt{wo_key}{it}",
                                                       bufs=3))
                sbp = atx.enter_context(tc.tile_pool(name=f"sb{wo_key}{it}",
                                                     bufs=2))
                pv = [pvp.tile([128, TQ], BF, tag=f"pv{i}", name=f"pv{i}")
                      for i in range(NT)]
                for h_ in range(H):
                    ct, off = h_ // 2, (h_ % 2) * 64
                    av = avps.tile([65, 512], F32, tag="av", name="av")
                    pend = None

                    def emit_av(Pt, p):
                        c0p = 128 * p if causal else 0
                        nc.tensor.matmul(av[:, c0p:512],
                                         vtt[p][:, :, h_, 0:65],
                                         Pt[:, :, c0p:512],
                                         start=(p == 0), stop=(p == 3),
                                         perf_mode=DR)

                    for p in range(4):
                        Pt = ppool.tile([128, 2, TQ], F8, tag="P", name="P")
                        for s in range(2):
                            kt = 2 * p + s
                            c0 = 64 * kt if causal else 0
                            sp = pps.tile([128, 512], F32, tag="pp",
                                          name="pp")
                            nc.tensor.matmul(
                                sp[:, c0:512],
                                kTt[ct][off:off + 64,
                                        128 * kt:128 * (kt + 1)],
                                qTt[ct][off:off + 64, c0:512],
                                start=True, stop=True)
                            nc.scalar.activation(Pt[:, s, c0:512],
                                                 sp[:, c0:512], AF.Exp,
                                                 bias=expb[:], scale=0.125)
                            if causal:
                                if s == 1:
                                    nc.gpsimd.memset(
                                        Pt[:, 1, 128 * p:128 * p + 64], 0.0)
                                with nc.allow_low_precision(reason="fp8 P"):
                                    nc.vector.tensor_mul(
                                        Pt[:, s, c0:c0 + 64],
                                        Pt[:, s, c0:c0 + 64], tri_sb[:])
                        if pend is not None:
                            emit_av(*pend)
                        pend = (Pt, p)
                    emit_av(*pend)
                    rinv = sbp.tile([1, 512], F32R, tag="ri", name="ri")
                    with nc.allow_low_precision(reason="f32r rinv"):
                        nc.vector.reciprocal(rinv[:], av[64:65, :])
                    rb_ps = rbps.tile([128, 512], F32, tag="rb", name="rb")
                    nc.tensor.matmul(rb_ps[:], ones1[:], rinv[:],
                                     start=True, stop=True)
                    rb = sbp.tile([64, 512], F32, tag="rs", name="rs")
                    nc.vector.tensor_copy(rb[:], rb_ps[0:64, :])
                    with nc.allow_low_precision(reason="bf16 pv"):
                        nc.vector.tensor_mul(pv[ct][off:off + 64, :],
                                             av[0:64, :], rb[:])
                ln_start_stats(lnst, avps, "av")
                for co in range(NT):
                    wt = wpool.tile([128, NT, 128], BF, tag="wo", name="wo",
                                    bufs=2)
                    nc.sync.dma_start(wt[:], wq_ap(t[wo_key], co))
                    ps = ops.tile([128, 512], F32, tag="o", name="o")
                    for cc in range(NT):
                        nc.tensor.matmul(ps[:], wt[:, cc, :], pv[cc][:],
                                         start=(cc == 0), stop=(cc == NT - 1))
                    with nc.allow_low_precision(reason="bf16 y"):
                        nc.vector.tensor_add(out_y[co][:], ps[:], resid(co))
                    ln_feed(lnst, out_y[co], co)

        # ================= main flow =================
        srcp = ctx.enter_context(tc.tile_pool(name=f"sp{it}", bufs=1))
        qpool = ctx.enter_context(tc.tile_pool(name=f"qp{it}", bufs=1))

        src_t = srcp.tile([128, NT, T], F8, tag="src", name="src")
        xl8_t = srcp.tile([128, NT, TQ], F8, tag="xl8", name="xl8")
        xlbf_t = srcp.tile([128, NT, TQ], BF, tag="xlbf", name="xlbf")
        enc_t = srcp.tile([128, NT, T], F8, tag="enc", name="enc")

        with ExitStack() as actx:
            qkv = actx.enter_context(tc.tile_pool(name=f"qkv{it}", bufs=1))
            pps = actx.enter_context(tc.tile_pool(
                name=f"ps{it}", bufs=3, space="PSUM"))
            avps = actx.enter_context(tc.tile_pool(
                name=f"as{it}", bufs=2, space="PSUM"))
            rbps = actx.enter_context(tc.tile_pool(
                name=f"rs{it}", bufs=1, space="PSUM"))
            ops = actx.enter_context(tc.tile_pool(
                name=f"os{it}", bufs=2, space="PSUM"))

            def new_kv(pfx):
                k = [qkv.tile([128, T], BF, tag=f"{pfx}k{i}", name=f"k{i}")
                     for i in range(NT)]
                v = [qkv.tile([128, 2, H, 65], F8, tag=f"{pfx}v{i}",
                              name=f"v{i}") for i in range(4)]
                return k, v

            qT = [qpool.tile([128, TQ], BF, tag=f"q{i}", name=f"q{i}")
                  for i in range(NT)]
            y18_t = qpool.tile([128, NT, TQ], F8, tag="y18", name="y18")
            qT2 = [qpool.tile([128, TQ], BF, tag=f"q2{i}", name=f"q2{i}")
                   for i in range(NT)]
            x1t = [qpool.tile([128, TQ], BF, tag=f"x1{i}", name=f"x1{i}")
                   for i in range(NT)]

            # ---- self-attention ----
            kTt, vtt = new_kv("s")
            with ExitStack() as sctx:
                for jp in range(4):
                    nc.sync.dma_start(xl8_t[:, 2 * jp:2 * jp + 2, :],
                                      chunk_ap(t["xl8"], TQ, jp))
                    nc.sync.dma_start(src_t[:, 2 * jp:2 * jp + 2, :],
                                      chunk_ap(t["xT8"], T, jp))
                qkv_proj(xl8_t, "wq18", TQ,
                         lambda ot, q0, ps: nc.scalar.activation(
                             qT[ot][:], ps[:], AF.Identity, scale=cQK), pps)

                def k_out(kTt_):
                    def f(ot, q0, ps):
                        with nc.allow_low_precision(reason="bf16 K"):
                            nc.vector.tensor_scalar_mul(
                                kTt_[ot][:, q0:q0 + 512], ps[:], cQK)
                    return f

                qkv_proj(src_t, "wk18", T, k_out(kTt), pps)
                load_main_consts()
                v_proj(src_t, "wv18", vtt, "pad18", "pad1s", pps)
                for jp in range(4):
                    nc.gpsimd.dma_start(xlbf_t[:, 2 * jp:2 * jp + 2, :],
                                        chunk_ap(t["xl_bf"], TQ, jp))
                    nc.gpsimd.dma_start(enc_t[:, 2 * jp:2 * jp + 2, :],
                                        chunk_ap(t["encT8"], T, jp))
                lnst1 = ln_begin(sctx, "s")
                attention(qT, kTt, vtt, "wo1bf",
                          lambda co: xlbf_t[:, co, :], qT, True,
                          pps, avps, rbps, ops, lnst1)
                for co in range(NT):
                    nc.scalar.activation(y18_t[:, co, :], qT[co][:],
                                         AF.Identity, scale=SY1)
                # K2/V2 fill the LN1-chain bubble on the PE
                kTt2, vtt2 = new_kv("c")
                qkv_proj(enc_t, "wk28", T, k_out(kTt2), pps)
                fold1 = ln_finish(lnst1, qT, lv["g1"], lv["b1"], x1t,
                                  fold_c=cQ2)
                v_proj(enc_t, "wv28", vtt2, "pad28", "pad2s", pps)
                # Q2 with LN1 folded through Wq2
                for ot in range(NT):
                    wt = wpool.tile([128, NT, 128], F8, tag="w", name="w")
                    nc.sync.dma_start(wt[:], wq_ap(t["wq28"], ot))
                    ps = pps.tile([128, 512], F32, tag="pp", name="pp")
                    for jp in range(4):
                        nc.tensor.matmul(
                            ps[:], wt[:, 2 * jp:2 * jp + 2, :],
                            y18_t[:, 2 * jp:2 * jp + 2, :],
                            start=(jp == 0), stop=(jp == 3), perf_mode=DR)
                    fold_epilogue(ps, fold1, lv["csq2"], ot, qT2[ot][:],
                                  AF.Identity, lv["bq2"], 1.0)

            # ---- cross-attention ----
            with ExitStack() as cctx:
                lnst2 = ln_begin(cctx, "c")
                attention(qT2, kTt2, vtt2, "wo2bf",
                          lambda co: x1t[co][:], qT2, False,
                          pps, avps, rbps, ops, lnst2)
                y2a_t = qpool.tile([128, NT, TQ], F8, tag="y2a", name="y2a")
                y2b_t = qpool.tile([128, NT, TQ], F8, tag="y2b", name="y2b")
                for co in range(NT):
                    nc.scalar.activation(y2a_t[:, co, :], qT2[co][:],
                                         AF.Identity, scale=SY2)
                    with nc.allow_low_precision(reason="fp8 y2b"):
                        nc.gpsimd.scalar_tensor_tensor(
                            out=y2b_t[:, co, :], in0=qT2[co][:], scalar=SY2,
                            in1=y2a_t[:, co, :], op0=ALU.mult,
                            op1=ALU.subtract)
                fold2 = ln_finish(lnst2, qT2, lv["g2"], lv["b2"], x1t,
                                  fold_c=cF1)

        # ---- FFN + AddNorm ----  (x1t now holds x2 = LN2 out)
        with ExitStack() as fctx:
            y3p = fctx.enter_context(tc.tile_pool(name=f"y3{it}", bufs=1))
            y3 = [y3p.tile([128, TQ], F32, tag=f"z{i}", name=f"z{i}")
                  for i in range(NT)]
            h8_t = y3p.tile([128, FT, TQ], F8, tag="h8", name="h8")
            lnps3 = fctx.enter_context(tc.tile_pool(
                name=f"l3{it}", bufs=2, space="PSUM"))
            lnst3 = ln_begin(fctx, "f", lnps3, "ln")
            with ExitStack() as mctx:
                w1p = mctx.enter_context(tc.tile_pool(name=f"w1{it}", bufs=4))
                w2p = mctx.enter_context(tc.tile_pool(name=f"w2{it}", bufs=2))
                pp1 = mctx.enter_context(tc.tile_pool(
                    name=f"f1{it}", bufs=3, space="PSUM"))
                pp2 = mctx.enter_context(tc.tile_pool(
                    name=f"f2{it}", bufs=2, space="PSUM"))
                for fg in range(FT):
                    w1t = w1p.tile([128, 2, NT, 128], F8, tag="w1", name="w1")
                    nc.sync.dma_start(
                        w1t[:],
                        bass.AP(tensor=t["wf1ab"], offset=fg * 128 * 2 * C,
                                ap=[[2 * C, 128], [C, 2], [128, NT],
                                    [1, 128]]))
                    ps = pp1.tile([128, 512], F32, tag="p1", name="p1")
                    for term, (ab, ysrc) in enumerate(
                            ((0, y2a_t), (1, y2a_t), (0, y2b_t))):
                        for jp in range(4):
                            nc.tensor.matmul(
                                ps[:], w1t[:, ab, 2 * jp:2 * jp + 2, :],
                                ysrc[:, 2 * jp:2 * jp + 2, :],
                                start=(term == 0 and jp == 0),
                                stop=(term == 2 and jp == 3), perf_mode=DR)
                    fold_epilogue(ps, fold2, lv["csf1"], fg,
                                  h8_t[:, fg, :], AF.Relu, lv["bf1s"], SH)
                for fb in range(2):
                    for co in range(NT):
                        w2t = w2p.tile([128, 2, 16, 128], F8, tag="w2",
                                       name="w2")
                        nc.sync.dma_start(
                            w2t[:],
                            bass.AP(tensor=t["wf2ab"],
                                    offset=co * 128 * 8192 + fb * 2048,
                                    ap=[[8192, 128], [4096, 2], [128, 16],
                                        [1, 128]]))
                        ps = pp2.tile([128, 512], F32, tag="p2", name="p2")
                        for ab in range(2):
                            for jp in range(8):
                                nc.tensor.matmul(
                                    ps[:], w2t[:, ab, 2 * jp:2 * jp + 2, :],
                                    h8_t[:, 16 * fb + 2 * jp:
                                         16 * fb + 2 * jp + 2, :],
                                    start=(ab == 0 and jp == 0),
                                    stop=(ab == 1 and jp == 7), perf_mode=DR)
                        if fb == 0:
                            nc.vector.tensor_scalar_mul(y3[co][:], ps[:], cF2)
                        else:
                            nc.vector.scalar_tensor_tensor(
                                out=y3[co][:], in0=ps[:], scalar=cF2,
                                in1=y3[co][:], op0=ALU.mult, op1=ALU.add)
                            nc.vector.scalar_tensor_tensor(
                                out=y3[co][:], in0=y3[co][:],
                                scalar=lv["bf2s"][:, co:co + 1],
                                in1=x1t[co][:], op0=ALU.add, op1=ALU.add)
                            ln_feed(lnst3, y3[co], co, f32=True)
            out_f = [y3p.tile([128, TQ], F32, tag=f"of{i}", name=f"of{i}")
                     for i in range(NT)]
            ln_finish_chunked(lnst3, y3, lv["g3"], lv["b3"], out_f, nch=4)
            for cch in range(4):
                sl = slice(cch * (TQ // 4), (cch + 1) * (TQ // 4))
                for co in range(NT):
                    nc.sync.dma_start(
                        t["outT"][128 * co:128 * (co + 1), sl],
                        out_f[co][:, sl])


def _q8(x, s):
    import ml_dtypes
    y = np.asarray(np.asarray(x, np.float32) * np.float32(s),
                   ml_dtypes.float8_e4m3)
    assert np.isfinite(y.astype(np.float32)).all()
    return y


def _tile_w(w):
    """[cin, cout] -> [cout//128, 128, cin]: block[ot][p][128*ct+c] =
    w[128*ct+c, 128*ot+... block[ot, p, k] = w[k, 128*ot+p] is WRONG;
    stationary wants block[ot, p, k] such that tile[p, ct, c] =
    w[128*ct+p, 128*ot+c], i.e. block[ot, p, ct*128+c]."""
    cin, cout = w.shape
    return np.ascontiguousarray(
        w.reshape(cin // 128, 128, cout // 128, 128)
        .transpose(2, 1, 0, 3).reshape(cout // 128, 128, cin))


def _shard(inputs):
    import ml_dtypes
    F8NP = ml_dtypes.float8_e4m3
    BFNP = ml_dtypes.bfloat16
    x = np.asarray(inputs["x"], dtype=np.float32)
    enc = np.asarray(inputs["enc_out"], dtype=np.float32)
    tpad = np.asarray(inputs["tgt_pad_mask"]).astype(np.float32)
    spad = np.asarray(inputs["src_pad_mask"]).astype(np.float32)
    ws = {k: np.asarray(inputs[k], dtype=np.float32)
          for k in ("Wq1", "Wk1", "Wv1", "Wo1", "Wq2", "Wk2", "Wv2", "Wo2",
                    "Wf1", "Wf2")}
    lnv = {k: np.asarray(inputs[k], dtype=np.float32)
           for k in ("ln1_g", "ln1_b", "ln2_g", "ln2_b", "ln3_g", "ln3_b",
                     "bf1", "bf2")}
    cV = SV / (SX * SW)

    w8 = {}
    for src, dst in (("Wq1", "wq18"), ("Wk1", "wk18"), ("Wk2", "wk28")):
        w8[dst] = _tile_w(_q8(ws[src], SW))
    w8["wv18"] = np.ascontiguousarray(_q8(ws["Wv1"], SW))
    w8["wv28"] = np.ascontiguousarray(_q8(ws["Wv2"], SW))
    wq2f = lnv["ln1_g"][:, None] * ws["Wq2"]
    wq2q = _q8(wq2f, SW)
    w8["wq28"] = _tile_w(wq2q)
    csq2 = np.ascontiguousarray(wq2q.astype(np.float32).sum(axis=0))
    bq2 = np.ascontiguousarray(lnv["ln1_b"] @ ws["Wq2"])
    wo1bf = np.ascontiguousarray(_tile_w(ws["Wo1"]).astype(BFNP))
    wo2bf = np.ascontiguousarray(_tile_w(ws["Wo2"]).astype(BFNP))
    wf1f = lnv["ln2_g"][:, None] * ws["Wf1"]
    w1a = _q8(wf1f, SW)
    w1b = _q8(wf1f * SW - w1a.astype(np.float32), 1.0)
    csf1 = np.ascontiguousarray(
        (w1a.astype(np.float32) + w1b.astype(np.float32)).sum(axis=0))
    wf1ab = np.ascontiguousarray(np.concatenate(
        [_tile_w(w1a), _tile_w(w1b)], axis=2))          # [FT,128,2C]
    bf1s = np.ascontiguousarray(
        SH * (lnv["bf1"] + lnv["ln2_b"] @ ws["Wf1"]))
    w2a = _q8(ws["Wf2"], SW)
    w2b = _q8(ws["Wf2"] * SW - w2a.astype(np.float32), 1.0)
    ta, tb = _tile_w(w2a), _tile_w(w2b)                  # [8,128,4096]
    wf2ab = np.ascontiguousarray(
        np.stack([ta.reshape(NT, 128, 2, 2048),
                  tb.reshape(NT, 128, 2, 2048)], axis=2)
        .reshape(NT, 128, 2 * 2 * 2048))
    in_maps = []
    for b in range(B):
        xT8 = np.ascontiguousarray(_q8(x[b].T, SX))
        eT8 = np.ascontiguousarray(_q8(enc[b].T, SX))
        p1s = np.ascontiguousarray((1.0 - tpad[b]) * cV)
        p2s = np.ascontiguousarray((1.0 - spad[b]) * cV)
        p18 = np.ascontiguousarray((1.0 - tpad[b]).astype(F8NP))
        p28 = np.ascontiguousarray((1.0 - spad[b]).astype(F8NP))
        for h in range(2):
            xl = x[b, h::2, :].T
            trih = (np.arange(128)[:, None] <= 2 * np.arange(64)[None, :] + h
                    ).astype(F8NP)
            in_maps.append({
                "xT8": xT8, "xl8": np.ascontiguousarray(_q8(xl, SX)),
                "encT8": eT8,
                "xl_bf": np.ascontiguousarray(xl.astype(BFNP)),
                **w8, "wo1bf": wo1bf, "wo2bf": wo2bf,
                "wf1ab": wf1ab, "wf2ab": wf2ab,
                "csq2": csq2, "bq2": bq2, "csf1": csf1,
                "bf1s": bf1s, "bf2s": lnv["bf2"],
                "tri8": np.ascontiguousarray(trih),
                "pad18": p18, "pad28": p28,
                "pad1s": p1s, "pad2s": p2s,
                "g1": lnv["ln1_g"], "b1": lnv["ln1_b"],
                "g2": lnv["ln2_g"], "b2": lnv["ln2_b"],
                "g3": lnv["ln3_g"], "b3": lnv["ln3_b"],
            })
    return in_maps


def _get_nc(repeat=1):
    if repeat not in _CACHE:
        _CACHE[repeat] = _build(repeat)
    return _CACHE[repeat]


def kernel(**inputs):
    from concourse.bass_utils import run_bass_kernel_spmd
    nc = _get_nc()
    in_maps = _shard(inputs)
    res = run_bass_kernel_spmd(nc, in_maps, core_ids=list(range(8)))
    out = np.empty((B, T, C), np.float32)
    for core in range(8):
        b, h = core // 2, core % 2
        out[b, h::2, :] = res.results[core]["outT"].T
    return out


# revision 16
# speedup vs baseline: 1.2942x; 1.0488x over previous
"""Decoder block kernel for 8 Trainium2 NeuronCores — fp8/bf16 version.

Sharding: core = 2*b + h handles batch b, query tokens q with q % 2 == h
(interleaved so the causal-mask structure is identical on every core).

Precision plan (validated against the fp32 reference in numpy, max rel
err ~1.2e-2 vs the 2e-2 gate):
- Q/K/V projections: fp8e4m3 DoubleRow (both operands fp8, contraction
  pairs packed), 4x the bf16 matmul rate.  Activations and weights are
  pre-scaled by powers of two so fp8's 3-bit mantissa covers them; the
  scales fold into existing epilogue constants.
- Attention scores: bf16 (the d=64 contraction can't pack into
  DoubleRow partitions without a transpose).  True causal lower bound
  per key tile (bf16 runs full rate at any free size).
- P = exp(score/8)*SP written fp8 by the activation op (bias=ln SP);
  AV in fp8 DoubleRow over key-tile pairs; the pad column of V doubles
  as the softmax denominator accumulator.
- O projections: bf16 (error budget).
- FFN1: fp8 DoubleRow with both operands 2-term same-scale splits
  (W ~ (A+B)/SW, x2 split on device) -> 3 accumulation passes into one
  psum group.  FFN2: weights split only (h single-fp8).
- LayerNorm statistics via ones-matmuls on bf16; LN1/LN2 affines folded
  through Wq2/Wf1 (colsums of the *quantized* weights for exact mean
  cancellation).
"""
import numpy as np

B, T, C, H, D, FF = 4, 1024, 1024, 16, 64, 4096
NT = C // 128   # 8 partition tiles of the model dim
KT = T // 128   # 8 context-token tiles
FT = FF // 128  # 32
TQ = T // 2     # 512 local query tokens per core

SW = 1024.0     # weight fp8 scale
SX = 16.0       # x / enc fp8 scale
SV = 32.0       # V fp8 scale
SP = 8.0        # P fp8 scale
SY1 = 16.0      # y1 (pre-LN1) fp8 scale
SY2 = 16.0      # y2 (pre-LN2) fp8 scale
SH = 32.0       # h (FFN hidden) fp8 scale

_CACHE = {}


def _build(repeat=1):
    import concourse.bacc as bacc
    import concourse.tile as tile
    from concourse import mybir

    nc = bacc.Bacc(None, target_bir_lowering=False)
    F32 = mybir.dt.float32
    F8 = mybir.dt.float8e4
    BF = mybir.dt.bfloat16

    t = {}

    def din(name, shape, dt=F8):
        t[name] = nc.dram_tensor(name, shape, dt, kind="ExternalInput")

    din("xT8", [C, T])
    din("xl8", [C, TQ])
    din("encT8", [C, T])
    din("xl_bf", [C, TQ], BF)
    # stationary weights pre-tiled [n_out_blocks, 128, cin]
    for k in ("wq18", "wk18", "wq28", "wk28"):
        din(k, [NT, 128, C])
    # V weights raw [cin, cout] (moving operand)
    din("wv18", [C, C])
    din("wv28", [C, C])
    din("wo1bf", [NT, 128, C], BF)
    din("wo2bf", [NT, 128, C], BF)
    din("wf1ab", [FT, 128, 2 * C])          # [fg][p][ab*C + k]
    din("wf2ab", [NT, 128, 2 * 2 * 2048])   # [co][p][ab][fb][2048]
    din("tri8", [128, 64])
    din("pad18", [T])
    din("pad28", [T])
    din("csq2", [C], F32)
    din("bq2", [C], F32)
    din("pad1s", [T], F32)
    din("pad2s", [T], F32)
    din("csf1", [FF], F32)
    din("bf1s", [FF], F32)
    din("bf2s", [C], F32)
    for k in ("g1", "b1", "g2", "b2", "g3", "b3"):
        din(k, [C], F32)
    t["outT"] = nc.dram_tensor("outT", [C, TQ], F32, kind="ExternalOutput")

    with tile.TileContext(nc) as tc:
        for it in range(repeat):
            _emit(nc, tc, t, it)
    nc.compile()
    return nc


def _emit(nc, tc, t, it):
    from contextlib import ExitStack
    import math
    import concourse.bass as bass
    from concourse import mybir

    F32 = mybir.dt.float32
    F32R = mybir.dt.float32r
    F8 = mybir.dt.float8e4
    BF = mybir.dt.bfloat16
    AF = mybir.ActivationFunctionType
    ALU = mybir.AluOpType
    DR = mybir.MatmulPerfMode.DoubleRow

    cQK = 1.0 / (SX * SW)        # Q/K psum -> bf16
    cQ2 = 1.0 / (SY1 * SW)       # Q2 fold rstd scale
    cF1 = 1.0 / (SY2 * SW)       # FFN1 fold rstd scale
    cF2 = 1.0 / (SH * SW)        # FFN2 psum -> value

    def vec_ap(dram, n):
        return bass.AP(tensor=dram, offset=0, ap=[[1, 128], [128, n // 128]])

    def wq_ap(wdram, ot):
        """Pre-tiled stationary block ot: [128, NT, 128]."""
        return bass.AP(tensor=wdram, offset=ot * 128 * C,
                       ap=[[C, 128], [128, NT], [1, 128]])

    def chunk_ap(dram, n, jp):
        """ct-pair chunk of a [C, n] tensor: [128, 2, n]."""
        return bass.AP(tensor=dram, offset=256 * jp * n,
                       ap=[[n, 128], [128 * n, 2], [1, n]])

    with ExitStack() as ctx:
        consts = ctx.enter_context(tc.tile_pool(name=f"con{it}", bufs=1))
        ones128 = consts.tile([128, 128], BF, tag="o128", name="o128")
        nc.vector.memset(ones128[:], 1.0)
        ones128r = consts.tile([128, 128], F32R, tag="o128r", name="o128r")
        nc.vector.memset(ones128r[:].bitcast(F32), 1.0)
        ones1 = consts.tile([1, 128], F32R, tag="o1", name="o1")
        nc.vector.memset(ones1[:].bitcast(F32), 1.0 / SV)
        eps_t = consts.tile([128, 1], F32, tag="eps", name="eps")
        nc.vector.memset(eps_t[:], 1e-5)
        expb = consts.tile([128, 1], F32, tag="expb", name="expb")
        nc.vector.memset(expb[:], math.log(SP))
        tri_sb = consts.tile([128, 64], F8, tag="tri", name="tri")

        def ldvec(key, n):
            s = consts.tile([128, n // 128], F32, tag=key, name=key)
            nc.gpsimd.dma_start(s[:], vec_ap(t[key], n))
            return s

        lv = {}

        def load_main_consts():
            for key, n in (("g1", C), ("b1", C), ("g2", C), ("b2", C),
                           ("g3", C), ("b3", C), ("csq2", C), ("bq2", C),
                           ("csf1", FF), ("bf1s", FF), ("bf2s", C)):
                lv[key] = ldvec(key, n)
            nc.gpsimd.dma_start(tri_sb[:], t["tri8"][:])

        pad_sb = {"pad1s": ldvec("pad1s", T), "pad2s": ldvec("pad2s", T)}

        wpool = ctx.enter_context(tc.tile_pool(name=f"wp{it}", bufs=4))
        foldp = ctx.enter_context(tc.tile_pool(name=f"fp{it}", bufs=2))
        statp = ctx.enter_context(tc.tile_pool(name=f"st{it}", bufs=1))

        # ---------- LN helpers ----------
        def ln_begin(sctx, tagp, lnps=None, lntag=None):
            sqp = sctx.enter_context(tc.tile_pool(name=f"sq{tagp}{it}", bufs=2))
            scr = sctx.enter_context(tc.tile_pool(name=f"ls{tagp}{it}", bufs=1))
            st = {"sqp": sqp, "scr": scr, "ps1": None, "ps2": None}
            if lnps is not None:
                ln_start_stats(st, lnps, lntag)
            return st

        def ln_start_stats(st, lnps, lntag):
            st["ps1"] = lnps.tile([128, TQ], F32, tag=lntag, name="ln")
            st["ps2"] = lnps.tile([128, TQ], F32, tag=lntag, name="ln")

        def ln_feed(st, y_tile, ct, f32=False):
            sq = st["sqp"].tile([128, TQ], BF, tag="sq", name="sq")
            with nc.allow_low_precision(reason="bf16 stats ok"):
                nc.vector.tensor_mul(sq[:], y_tile[:], y_tile[:])
            y_ap = y_tile[:].bitcast(F32R) if f32 else y_tile[:]
            ones = ones128r if f32 else ones128
            nc.tensor.matmul(st["ps1"][:], ones[:], y_ap,
                             start=(ct == 0), stop=(ct == NT - 1),
                             skip_group_check=True)
            nc.tensor.matmul(st["ps2"][:], ones128[:], sq[:],
                             start=(ct == 0), stop=(ct == NT - 1),
                             skip_group_check=True)

        def ln_finish(st, y_in, g, b, out_t, fold_c=None):
            """Stats; normalized+affine out_t (bf16).  With fold_c also
            produce (r2, un): r2 = rstd*fold_c, un = -m*rstd/SW."""
            scr = st["scr"]
            m = scr.tile([128, TQ], F32, tag="m", name="m")
            nc.vector.tensor_scalar_mul(m[:], st["ps1"][:], 1.0 / C)
            ms = scr.tile([128, TQ], F32, tag="v", name="v")
            nc.vector.tensor_scalar_mul(ms[:], st["ps2"][:], 1.0 / C)
            m2 = scr.tile([128, TQ], F32, tag="v2", name="v2")
            nc.vector.tensor_mul(m2[:], m[:], m[:])
            nc.vector.tensor_sub(ms[:], ms[:], m2[:])
            nc.scalar.activation(ms[:], ms[:], AF.Sqrt, bias=eps_t[:])
            rstd = scr.tile([128, TQ], F32, tag="r", name="r")
            nc.vector.reciprocal(rstd[:], ms[:])
            ret = None
            if fold_c is not None:
                un = statp.tile([128, TQ], F32, tag="un", name="un")
                nc.vector.tensor_mul(un[:], m[:], rstd[:])
                nc.vector.tensor_scalar(un[:], un[:], -1.0 / SW, None,
                                        op0=ALU.mult)
                r2 = statp.tile([128, TQ], F32, tag="r2", name="r2")
                nc.vector.tensor_scalar_mul(r2[:], rstd[:], fold_c)
                ret = (r2, un)
            for ct in range(NT):
                eng = nc.vector if ct % 2 == 0 else nc.gpsimd
                d = scr.tile([128, TQ], F32, tag="d", name="d", bufs=2)
                eng.tensor_sub(d[:], y_in[ct][:], m[:])
                eng.tensor_mul(d[:], d[:], rstd[:])
                with nc.allow_low_precision(reason="bf16 ln out"):
                    eng.tensor_scalar(out_t[ct][:], d[:], g[:, ct:ct + 1],
                                      b[:, ct:ct + 1], op0=ALU.mult,
                                      op1=ALU.add)
            return ret

        def ln_finish_chunked(st, y_in, g, b, out_t, nch=4):
            scr = st["scr"]
            m = scr.tile([128, TQ], F32, tag="m", name="m")
            ms = scr.tile([128, TQ], F32, tag="v", name="v")
            m2 = scr.tile([128, TQ], F32, tag="v2", name="v2")
            rstd = scr.tile([128, TQ], F32, tag="r", name="r")
            w = TQ // nch
            for cch in range(nch):
                sl = slice(cch * w, (cch + 1) * w)
                nc.vector.tensor_scalar_mul(m[:, sl], st["ps1"][:, sl], 1.0 / C)
                nc.vector.tensor_scalar_mul(ms[:, sl], st["ps2"][:, sl], 1.0 / C)
                nc.vector.tensor_mul(m2[:, sl], m[:, sl], m[:, sl])
                nc.vector.tensor_sub(ms[:, sl], ms[:, sl], m2[:, sl])
                nc.scalar.activation(ms[:, sl], ms[:, sl], AF.Sqrt,
                                     bias=eps_t[:])
                nc.vector.reciprocal(rstd[:, sl], ms[:, sl])
                for ct in range(NT):
                    eng = nc.vector if ct % 2 == 0 else nc.gpsimd
                    d = scr.tile([128, w], F32, tag="dc", name="dc", bufs=3)
                    eng.tensor_sub(d[:], y_in[ct][:, sl], m[:, sl])
                    eng.tensor_mul(d[:], d[:], rstd[:, sl])
                    eng.tensor_scalar(out_t[ct][:, sl], d[:], g[:, ct:ct + 1],
                                      b[:, ct:ct + 1], op0=ALU.mult,
                                      op1=ALU.add)

        def fold_epilogue(ps, fold, cs, ot, out_ap, func, bias_sb, scale):
            """out = func(scale*(r2*ps + un*cs[ot]) + bias[ot])"""
            r2, un = fold
            eng = nc.vector if ot % 2 == 0 else nc.gpsimd
            ftile = foldp.tile([128, TQ], F32, tag="ft", name="ft")
            eng.tensor_mul(ftile[:], ps[:], r2[:])
            eng.scalar_tensor_tensor(
                out=ftile[:], in0=un[:], scalar=cs[:, ot:ot + 1], in1=ftile[:],
                op0=ALU.mult, op1=ALU.add)
            nc.scalar.activation(out_ap, ftile[:], func,
                                 bias=bias_sb[:, ot:ot + 1], scale=scale)

        # ---------- projection helpers ----------
        def qkv_proj(src8, w8_key, n, out_fn, pp):
            for ot in range(NT):
                wt = wpool.tile([128, NT, 128], F8, tag="w", name="w")
                nc.sync.dma_start(wt[:], wq_ap(t[w8_key], ot))
                for q0 in range(0, n, 512):
                    ps = pp.tile([128, 512], F32, tag="pp", name="pp")
                    for jp in range(4):
                        nc.tensor.matmul(
                            ps[:], wt[:, 2 * jp:2 * jp + 2, :],
                            src8[:, 2 * jp:2 * jp + 2, q0:q0 + 512],
                            start=(jp == 0), stop=(jp == 3), perf_mode=DR)
                    out_fn(ot, q0, ps)

        def v_proj(src8, wv_key, vtt, pad8key, padskey, pp):
            wvq = []
            for half in range(2):
                wvt = wpool.tile([128, NT, 512], F8, tag="wv", name="wv",
                                 bufs=2)
                nc.sync.dma_start(
                    wvt[:], bass.AP(tensor=t[wv_key], offset=512 * half,
                                    ap=[[C, 128], [128 * C, NT], [1, 512]]))
                wvq.append(wvt)
            for p in range(4):
                for s in range(2):
                    tt = 2 * p + s
                    nc.scalar.dma_start(
                        vtt[p][:, s, :, 64:65],
                        bass.AP(tensor=t[pad8key], offset=128 * tt,
                                ap=[[1, 128], [0, H], [0, 1]]))
                    for half in range(2):
                        ps = pp.tile([128, 512], F32, tag="pp", name="pp")
                        for jp in range(4):
                            nc.tensor.matmul(
                                ps[:],
                                src8[:, 2 * jp:2 * jp + 2,
                                     128 * tt:128 * (tt + 1)],
                                wvq[half][:, 2 * jp:2 * jp + 2, :],
                                start=(jp == 0), stop=(jp == 3), perf_mode=DR)
                        veng = nc.vector if half == 0 else nc.gpsimd
                        with nc.allow_low_precision(reason="fp8 V"):
                            veng.tensor_scalar_mul(
                                vtt[p][:, s, 8 * half:8 * (half + 1), 0:64],
                                ps[:].rearrange("p (h d) -> p h d", d=64),
                                pad_sb[padskey][:, tt:tt + 1])

        def attention(qTt, kTt, vtt, wo_key, resid, out_y, causal,
                      pps, avps, rbps, ops, lnst, post_co=None):
            with ExitStack() as atx:
                pvp = atx.enter_context(tc.tile_pool(name=f"pv{wo_key}{it}",
                                                     bufs=1))
                ppool = atx.enter_context(tc.tile_pool(name=f"pt{wo_key}{it}",
                                                       bufs=3))
                sbp = atx.enter_context(tc.tile_pool(name=f"sb{wo_key}{it}",
                                                     bufs=2))
                pv = [pvp.tile([128, TQ], BF, tag=f"pv{i}", name=f"pv{i}")
                      for i in range(NT)]

                def rb_tail(av, rinv, ct, off):
                    rb_ps = rbps.tile([128, 512], F32, tag="rb", name="rb")
                    nc.tensor.matmul(rb_ps[:], ones1[:], rinv[:],
                                     start=True, stop=True)
                    rb = sbp.tile([64, 512], F32, tag="rs", name="rs")
                    nc.vector.tensor_copy(rb[:], rb_ps[0:64, :])
                    with nc.allow_low_precision(reason="bf16 pv"):
                        nc.vector.tensor_mul(pv[ct][off:off + 64, :],
                                             av[0:64, :], rb[:])

                pend_rb = None
                for h_ in range(H):
                    ct, off = h_ // 2, (h_ % 2) * 64
                    av = avps.tile([65, 512], F32, tag="av", name="av")
                    pend = None

                    def emit_av(Pt, p):
                        c0p = 128 * p if causal else 0
                        nc.tensor.matmul(av[:, c0p:512],
                                         vtt[p][:, :, h_, 0:65],
                                         Pt[:, :, c0p:512],
                                         start=(p == 0), stop=(p == 3),
                                         perf_mode=DR)

                    for p in range(4):
                        Pt = ppool.tile([128, 2, TQ], F8, tag="P", name="P")
                        for s in range(2):
                            kt = 2 * p + s
                            c0 = 64 * kt if causal else 0
                            sp = pps.tile([128, 512], F32, tag="pp",
                                          name="pp")
                            nc.tensor.matmul(
                                sp[:, c0:512],
                                kTt[ct][off:off + 64,
                                        128 * kt:128 * (kt + 1)],
                                qTt[ct][off:off + 64, c0:512],
                                start=True, stop=True)
                            nc.scalar.activation(Pt[:, s, c0:512],
                                                 sp[:, c0:512], AF.Exp,
                                                 bias=expb[:], scale=0.125)
                            if causal:
                                if s == 1:
                                    nc.gpsimd.memset(
                                        Pt[:, 1, 128 * p:128 * p + 64], 0.0)
                                with nc.allow_low_precision(reason="fp8 P"):
                                    nc.gpsimd.tensor_mul(
                                        Pt[:, s, c0:c0 + 64],
                                        Pt[:, s, c0:c0 + 64], tri_sb[:])
                        if pend is not None:
                            emit_av(*pend)
                        pend = (Pt, p)
                    emit_av(*pend)
                    rinv = sbp.tile([1, 512], F32R, tag="ri", name="ri")
                    with nc.allow_low_precision(reason="f32r rinv"):
                        nc.vector.reciprocal(rinv[:], av[64:65, :])
                    if pend_rb is not None:
                        rb_tail(*pend_rb)
                    pend_rb = (av, rinv, ct, off)
                rb_tail(*pend_rb)
                ln_start_stats(lnst, avps, "av")
                for co in range(NT):
                    wt = wpool.tile([128, NT, 128], BF, tag="wo", name="wo",
                                    bufs=2)
                    nc.sync.dma_start(wt[:], wq_ap(t[wo_key], co))
                    ps = ops.tile([128, 512], F32, tag="o", name="o")
                    for cc in range(NT):
                        nc.tensor.matmul(ps[:], wt[:, cc, :], pv[cc][:],
                                         start=(cc == 0), stop=(cc == NT - 1))
                    with nc.allow_low_precision(reason="bf16 y"):
                        nc.vector.tensor_add(out_y[co][:], ps[:], resid(co))
                    ln_feed(lnst, out_y[co], co)
                    if post_co is not None:
                        post_co(co)

        # ================= main flow =================
        srcp = ctx.enter_context(tc.tile_pool(name=f"sp{it}", bufs=1))
        qpool = ctx.enter_context(tc.tile_pool(name=f"qp{it}", bufs=1))

        src_t = srcp.tile([128, NT, T], F8, tag="src", name="src")
        xl8_t = srcp.tile([128, NT, TQ], F8, tag="xl8", name="xl8")
        xlbf_t = srcp.tile([128, NT, TQ], BF, tag="xlbf", name="xlbf")
        enc_t = srcp.tile([128, NT, T], F8, tag="enc", name="enc")

        with ExitStack() as actx:
            qkv = actx.enter_context(tc.tile_pool(name=f"qkv{it}", bufs=1))
            pps = actx.enter_context(tc.tile_pool(
                name=f"ps{it}", bufs=3, space="PSUM"))
            avps = actx.enter_context(tc.tile_pool(
                name=f"as{it}", bufs=2, space="PSUM"))
            rbps = actx.enter_context(tc.tile_pool(
                name=f"rs{it}", bufs=1, space="PSUM"))
            ops = actx.enter_context(tc.tile_pool(
                name=f"os{it}", bufs=2, space="PSUM"))

            def new_kv(pfx):
                k = [qkv.tile([128, T], BF, tag=f"{pfx}k{i}", name=f"k{i}")
                     for i in range(NT)]
                v = [qkv.tile([128, 2, H, 65], F8, tag=f"{pfx}v{i}",
                              name=f"v{i}") for i in range(4)]
                return k, v

            qT = [qpool.tile([128, TQ], BF, tag=f"q{i}", name=f"q{i}")
                  for i in range(NT)]
            y18_t = qpool.tile([128, NT, TQ], F8, tag="y18", name="y18")
            qT2 = [qpool.tile([128, TQ], BF, tag=f"q2{i}", name=f"q2{i}")
                   for i in range(NT)]
            x1t = [qpool.tile([128, TQ], BF, tag=f"x1{i}", name=f"x1{i}")
                   for i in range(NT)]

            # ---- self-attention ----
            kTt, vtt = new_kv("s")
            with ExitStack() as sctx:
                for jp in range(4):
                    nc.sync.dma_start(xl8_t[:, 2 * jp:2 * jp + 2, :],
                                      chunk_ap(t["xl8"], TQ, jp))
                qkv_proj(xl8_t, "wq18", TQ,
                         lambda ot, q0, ps: nc.scalar.activation(
                             qT[ot][:], ps[:], AF.Identity, scale=cQK), pps)
                for jp in range(4):
                    nc.sync.dma_start(src_t[:, 2 * jp:2 * jp + 2, :],
                                      chunk_ap(t["xT8"], T, jp))

                def k_out(kTt_):
                    def f(ot, q0, ps):
                        eng = nc.vector if (ot + q0 // 512) % 2 == 0 \
                            else nc.gpsimd
                        with nc.allow_low_precision(reason="bf16 K"):
                            eng.tensor_scalar_mul(
                                kTt_[ot][:, q0:q0 + 512], ps[:], cQK)
                    return f

                qkv_proj(src_t, "wk18", T, k_out(kTt), pps)
                load_main_consts()
                v_proj(src_t, "wv18", vtt, "pad18", "pad1s", pps)
                for jp in range(4):
                    nc.gpsimd.dma_start(xlbf_t[:, 2 * jp:2 * jp + 2, :],
                                        chunk_ap(t["xl_bf"], TQ, jp))
                    nc.gpsimd.dma_start(enc_t[:, 2 * jp:2 * jp + 2, :],
                                        chunk_ap(t["encT8"], T, jp))
                lnst1 = ln_begin(sctx, "s")

                def y18_quant(co):
                    nc.scalar.activation(y18_t[:, co, :], qT[co][:],
                                         AF.Identity, scale=SY1)

                attention(qT, kTt, vtt, "wo1bf",
                          lambda co: xlbf_t[:, co, :], qT, True,
                          pps, avps, rbps, ops, lnst1, post_co=y18_quant)
                # K2/V2 fill the LN1-chain bubble on the PE
                kTt2, vtt2 = new_kv("c")
                qkv_proj(enc_t, "wk28", T, k_out(kTt2), pps)
                fold1 = ln_finish(lnst1, qT, lv["g1"], lv["b1"], x1t,
                                  fold_c=cQ2)
                v_proj(enc_t, "wv28", vtt2, "pad28", "pad2s", pps)
                # Q2 with LN1 folded through Wq2
                for ot in range(NT):
                    wt = wpool.tile([128, NT, 128], F8, tag="w", name="w")
                    nc.sync.dma_start(wt[:], wq_ap(t["wq28"], ot))
                    ps = pps.tile([128, 512], F32, tag="pp", name="pp")
                    for jp in range(4):
                        nc.tensor.matmul(
                            ps[:], wt[:, 2 * jp:2 * jp + 2, :],
                            y18_t[:, 2 * jp:2 * jp + 2, :],
                            start=(jp == 0), stop=(jp == 3), perf_mode=DR)
                    fold_epilogue(ps, fold1, lv["csq2"], ot, qT2[ot][:],
                                  AF.Identity, lv["bq2"], 1.0)

            # ---- cross-attention ----
            with ExitStack() as cctx:
                lnst2 = ln_begin(cctx, "c")
                y2a_t = qpool.tile([128, NT, TQ], F8, tag="y2a", name="y2a")
                y2b_t = qpool.tile([128, NT, TQ], F8, tag="y2b", name="y2b")

                def y2_quant(co):
                    nc.scalar.activation(y2a_t[:, co, :], qT2[co][:],
                                         AF.Identity, scale=SY2)
                    with nc.allow_low_precision(reason="fp8 y2b"):
                        nc.gpsimd.scalar_tensor_tensor(
                            out=y2b_t[:, co, :], in0=qT2[co][:], scalar=SY2,
                            in1=y2a_t[:, co, :], op0=ALU.mult,
                            op1=ALU.subtract)

                attention(qT2, kTt2, vtt2, "wo2bf",
                          lambda co: x1t[co][:], qT2, False,
                          pps, avps, rbps, ops, lnst2, post_co=y2_quant)
                fold2 = ln_finish(lnst2, qT2, lv["g2"], lv["b2"], x1t,
                                  fold_c=cF1)

        # ---- FFN + AddNorm ----  (x1t now holds x2 = LN2 out)
        with ExitStack() as fctx:
            y3p = fctx.enter_context(tc.tile_pool(name=f"y3{it}", bufs=1))
            y3 = [y3p.tile([128, TQ], F32, tag=f"z{i}", name=f"z{i}")
                  for i in range(NT)]
            h8_t = y3p.tile([128, FT, TQ], F8, tag="h8", name="h8")
            lnps3 = fctx.enter_context(tc.tile_pool(
                name=f"l3{it}", bufs=2, space="PSUM"))
            lnst3 = ln_begin(fctx, "f", lnps3, "ln")
            with ExitStack() as mctx:
                w1p = mctx.enter_context(tc.tile_pool(name=f"w1{it}", bufs=4))
                w2p = mctx.enter_context(tc.tile_pool(name=f"w2{it}", bufs=3))
                pp1 = mctx.enter_context(tc.tile_pool(
                    name=f"f1{it}", bufs=3, space="PSUM"))
                pp2 = mctx.enter_context(tc.tile_pool(
                    name=f"f2{it}", bufs=2, space="PSUM"))
                for fg in range(FT):
                    w1t = w1p.tile([128, 2, NT, 128], F8, tag="w1", name="w1")
                    nc.sync.dma_start(
                        w1t[:],
                        bass.AP(tensor=t["wf1ab"], offset=fg * 128 * 2 * C,
                                ap=[[2 * C, 128], [C, 2], [128, NT],
                                    [1, 128]]))
                    ps = pp1.tile([128, 512], F32, tag="p1", name="p1")
                    for term, (ab, ysrc) in enumerate(
                            ((0, y2a_t), (1, y2a_t), (0, y2b_t))):
                        for jp in range(4):
                            nc.tensor.matmul(
                                ps[:], w1t[:, ab, 2 * jp:2 * jp + 2, :],
                                ysrc[:, 2 * jp:2 * jp + 2, :],
                                start=(term == 0 and jp == 0),
                                stop=(term == 2 and jp == 3), perf_mode=DR)
                    fold_epilogue(ps, fold2, lv["csf1"], fg,
                                  h8_t[:, fg, :], AF.Relu, lv["bf1s"], SH)
                for fb in range(2):
                    for co in range(NT):
                        w2t = w2p.tile([128, 2, 16, 128], F8, tag="w2",
                                       name="w2")
                        nc.sync.dma_start(
                            w2t[:],
                            bass.AP(tensor=t["wf2ab"],
                                    offset=co * 128 * 8192 + fb * 2048,
                                    ap=[[8192, 128], [4096, 2], [128, 16],
                                        [1, 128]]))
                        ps = pp2.tile([128, 512], F32, tag="p2", name="p2")
                        for ab in range(2):
                            for jp in range(8):
                                nc.tensor.matmul(
                                    ps[:], w2t[:, ab, 2 * jp:2 * jp + 2, :],
                                    h8_t[:, 16 * fb + 2 * jp:
                                         16 * fb + 2 * jp + 2, :],
                                    start=(ab == 0 and jp == 0),
                                    stop=(ab == 1 and jp == 7), perf_mode=DR)
                        if fb == 0:
                            nc.vector.tensor_scalar_mul(y3[co][:], ps[:], cF2)
                        else:
                            nc.vector.scalar_tensor_tensor(
                                out=y3[co][:], in0=ps[:], scalar=cF2,
                                in1=y3[co][:], op0=ALU.mult, op1=ALU.add)
                            nc.vector.scalar_tensor_tensor(
                                out=y3[co][:], in0=y3[co][:],
                                scalar=lv["bf2s"][:, co:co + 1],
                                in1=x1t[co][:], op0=ALU.add, op1=ALU.add)
                            ln_feed(lnst3, y3[co], co, f32=True)
            out_f = [y3p.tile([128, TQ], F32, tag=f"of{i}", name=f"of{i}")
                     for i in range(NT)]
            ln_finish_chunked(lnst3, y3, lv["g3"], lv["b3"], out_f, nch=2)
            for cch in range(2):
                sl = slice(cch * (TQ // 2), (cch + 1) * (TQ // 2))
                for co in range(NT):
                    eng = nc.sync if co % 2 == 0 else nc.gpsimd
                    eng.dma_start(
                        t["outT"][128 * co:128 * (co + 1), sl],
                        out_f[co][:, sl])


def _q8(x, s):
    import ml_dtypes
    y = np.asarray(np.asarray(x, np.float32) * np.float32(s),
                   ml_dtypes.float8_e4m3)
    assert np.isfinite(y.astype(np.float32)).all()
    return y


def _tile_w(w):
    """[cin, cout] -> [cout//128, 128, cin]: block[ot][p][128*ct+c] =
    w[128*ct+c, 128*ot+... block[ot, p, k] = w[k, 128*ot+p] is WRONG;
    stationary wants block[ot, p, k] such that tile[p, ct, c] =
    w[128*ct+p, 128*ot+c], i.e. block[ot, p, ct*128+c]."""
    cin, cout = w.shape
    return np.ascontiguousarray(
        w.reshape(cin // 128, 128, cout // 128, 128)
        .transpose(2, 1, 0, 3).reshape(cout // 128, 128, cin))


def _shard(inputs):
    import ml_dtypes
    F8NP = ml_dtypes.float8_e4m3
    BFNP = ml_dtypes.bfloat16
    x = np.asarray(inputs["x"], dtype=np.float32)
    enc = np.asarray(inputs["enc_out"], dtype=np.float32)
    tpad = np.asarray(inputs["tgt_pad_mask"]).astype(np.float32)
    spad = np.asarray(inputs["src_pad_mask"]).astype(np.float32)
    ws = {k: np.asarray(inputs[k], dtype=np.float32)
          for k in ("Wq1", "Wk1", "Wv1", "Wo1", "Wq2", "Wk2", "Wv2", "Wo2",
                    "Wf1", "Wf2")}
    lnv = {k: np.asarray(inputs[k], dtype=np.float32)
           for k in ("ln1_g", "ln1_b", "ln2_g", "ln2_b", "ln3_g", "ln3_b",
                     "bf1", "bf2")}
    cV = SV / (SX * SW)

    w8 = {}
    for src, dst in (("Wq1", "wq18"), ("Wk1", "wk18"), ("Wk2", "wk28")):
        w8[dst] = _tile_w(_q8(ws[src], SW))
    w8["wv18"] = np.ascontiguousarray(_q8(ws["Wv1"], SW))
    w8["wv28"] = np.ascontiguousarray(_q8(ws["Wv2"], SW))
    wq2f = lnv["ln1_g"][:, None] * ws["Wq2"]
    wq2q = _q8(wq2f, SW)
    w8["wq28"] = _tile_w(wq2q)
    csq2 = np.ascontiguousarray(wq2q.astype(np.float32).sum(axis=0))
    bq2 = np.ascontiguousarray(lnv["ln1_b"] @ ws["Wq2"])
    wo1bf = np.ascontiguousarray(_tile_w(ws["Wo1"]).astype(BFNP))
    wo2bf = np.ascontiguousarray(_tile_w(ws["Wo2"]).astype(BFNP))
    wf1f = lnv["ln2_g"][:, None] * ws["Wf1"]
    w1a = _q8(wf1f, SW)
    w1b = _q8(wf1f * SW - w1a.astype(np.float32), 1.0)
    csf1 = np.ascontiguousarray(
        (w1a.astype(np.float32) + w1b.astype(np.float32)).sum(axis=0))
    wf1ab = np.ascontiguousarray(np.concatenate(
        [_tile_w(w1a), _tile_w(w1b)], axis=2))          # [FT,128,2C]
    bf1s = np.ascontiguousarray(
        SH * (lnv["bf1"] + lnv["ln2_b"] @ ws["Wf1"]))
    w2a = _q8(ws["Wf2"], SW)
    w2b = _q8(ws["Wf2"] * SW - w2a.astype(np.float32), 1.0)
    ta, tb = _tile_w(w2a), _tile_w(w2b)                  # [8,128,4096]
    wf2ab = np.ascontiguousarray(
        np.stack([ta.reshape(NT, 128, 2, 2048),
                  tb.reshape(NT, 128, 2, 2048)], axis=2)
        .reshape(NT, 128, 2 * 2 * 2048))
    in_maps = []
    for b in range(B):
        xT8 = np.ascontiguousarray(_q8(x[b].T, SX))
        eT8 = np.ascontiguousarray(_q8(enc[b].T, SX))
        p1s = np.ascontiguousarray((1.0 - tpad[b]) * cV)
        p2s = np.ascontiguousarray((1.0 - spad[b]) * cV)
        p18 = np.ascontiguousarray((1.0 - tpad[b]).astype(F8NP))
        p28 = np.ascontiguousarray((1.0 - spad[b]).astype(F8NP))
        for h in range(2):
            xl = x[b, h::2, :].T
            trih = (np.arange(128)[:, None] <= 2 * np.arange(64)[None, :] + h
                    ).astype(F8NP)
            in_maps.append({
                "xT8": xT8, "xl8": np.ascontiguousarray(_q8(xl, SX)),
                "encT8": eT8,
                "xl_bf": np.ascontiguousarray(xl.astype(BFNP)),
                **w8, "wo1bf": wo1bf, "wo2bf": wo2bf,
                "wf1ab": wf1ab, "wf2ab": wf2ab,
                "csq2": csq2, "bq2": bq2, "csf1": csf1,
                "bf1s": bf1s, "bf2s": lnv["bf2"],
                "tri8": np.ascontiguousarray(trih),
                "pad18": p18, "pad28": p28,
                "pad1s": p1s, "pad2s": p2s,
                "g1": lnv["ln1_g"], "b1": lnv["ln1_b"],
                "g2": lnv["ln2_g"], "b2": lnv["ln2_b"],
                "g3": lnv["ln3_g"], "b3": lnv["ln3_b"],
            })
    return in_maps


def _get_nc(repeat=1):
    if repeat not in _CACHE:
        _CACHE[repeat] = _build(repeat)
    return _CACHE[repeat]


def kernel(**inputs):
    from concourse.bass_utils import run_bass_kernel_spmd
    nc = _get_nc()
    in_maps = _shard(inputs)
    res = run_bass_kernel_spmd(nc, in_maps, core_ids=list(range(8)))
    out = np.empty((B, T, C), np.float32)
    for core in range(8):
        b, h = core // 2, core % 2
        out[b, h::2, :] = res.results[core]["outT"].T
    return out


# revision 17
# speedup vs baseline: 1.3398x; 1.0352x over previous
"""Decoder block kernel for 8 Trainium2 NeuronCores — fp8/bf16 version.

Sharding: core = 2*b + h handles batch b, query tokens q with q % 2 == h
(interleaved so the causal-mask structure is identical on every core).

Precision plan (validated against the fp32 reference in numpy, max rel
err ~1.2e-2 vs the 2e-2 gate):
- Q/K/V projections: fp8e4m3 DoubleRow (both operands fp8, contraction
  pairs packed), 4x the bf16 matmul rate.  Activations and weights are
  pre-scaled by powers of two so fp8's 3-bit mantissa covers them; the
  scales fold into existing epilogue constants.
- Attention scores: bf16 (the d=64 contraction can't pack into
  DoubleRow partitions without a transpose).  True causal lower bound
  per key tile (bf16 runs full rate at any free size).
- P = exp(score/8)*SP written fp8 by the activation op (bias=ln SP);
  AV in fp8 DoubleRow over key-tile pairs; the pad column of V doubles
  as the softmax denominator accumulator.
- O projections: bf16 (error budget).
- FFN1: fp8 DoubleRow with both operands 2-term same-scale splits
  (W ~ (A+B)/SW, x2 split on device) -> 3 accumulation passes into one
  psum group.  FFN2: weights split only (h single-fp8).
- LayerNorm statistics via ones-matmuls on bf16; LN1/LN2 affines folded
  through Wq2/Wf1 (colsums of the *quantized* weights for exact mean
  cancellation).
"""
import numpy as np

B, T, C, H, D, FF = 4, 1024, 1024, 16, 64, 4096
NT = C // 128   # 8 partition tiles of the model dim
KT = T // 128   # 8 context-token tiles
FT = FF // 128  # 32
TQ = T // 2     # 512 local query tokens per core

SW = 1024.0     # weight fp8 scale
SX = 16.0       # x / enc fp8 scale
SV = 32.0       # V fp8 scale
SP = 8.0        # P fp8 scale
SY1 = 16.0      # y1 (pre-LN1) fp8 scale
SY2 = 16.0      # y2 (pre-LN2) fp8 scale
SH = 32.0       # h (FFN hidden) fp8 scale

_CACHE = {}


def _build(repeat=1):
    import concourse.bacc as bacc
    import concourse.tile as tile
    from concourse import mybir

    nc = bacc.Bacc(None, target_bir_lowering=False)
    F32 = mybir.dt.float32
    F8 = mybir.dt.float8e4
    BF = mybir.dt.bfloat16

    t = {}

    def din(name, shape, dt=F8):
        t[name] = nc.dram_tensor(name, shape, dt, kind="ExternalInput")

    din("xT8", [C, T])
    din("xl8", [C, TQ])
    din("encT8", [C, T])
    din("xl_bf", [C, TQ], BF)
    # stationary weights pre-tiled [n_out_blocks, 128, cin]
    for k in ("wq18", "wk18", "wq28", "wk28"):
        din(k, [NT, 128, C])
    # V weights raw [cin, cout] (moving operand)
    din("wv18", [C, C])
    din("wv28", [C, C])
    din("wo1bf", [NT, 128, C], BF)
    din("wo2bf", [NT, 128, C], BF)
    din("wf1ab", [FT, 128, 2 * C])          # [fg][p][ab*C + k]
    din("wf2ab", [NT, 128, 2 * 2 * 2048])   # [co][p][ab][fb][2048]
    din("tri8", [128, 64])
    din("pad18", [T])
    din("pad28", [T])
    din("csq2", [C], F32)
    din("bq2", [C], F32)
    din("pad1s", [T], F32)
    din("pad2s", [T], F32)
    din("csf1", [FF], F32)
    din("bf1s", [FF], F32)
    din("bf2s", [C], F32)
    for k in ("g1", "b1", "g2", "b2", "g3", "b3"):
        din(k, [C], F32)
    t["outT"] = nc.dram_tensor("outT", [C, TQ], F32, kind="ExternalOutput")

    with tile.TileContext(nc) as tc:
        for it in range(repeat):
            _emit(nc, tc, t, it)
    nc.compile()
    return nc


def _emit(nc, tc, t, it):
    from contextlib import ExitStack
    import math
    import concourse.bass as bass
    from concourse import mybir

    F32 = mybir.dt.float32
    F32R = mybir.dt.float32r
    F8 = mybir.dt.float8e4
    BF = mybir.dt.bfloat16
    AF = mybir.ActivationFunctionType
    ALU = mybir.AluOpType
    DR = mybir.MatmulPerfMode.DoubleRow

    cQK = 1.0 / (SX * SW)        # Q/K psum -> bf16
    cQ2 = 1.0 / (SY1 * SW)       # Q2 fold rstd scale
    cF1 = 1.0 / (SY2 * SW)       # FFN1 fold rstd scale
    cF2 = 1.0 / (SH * SW)        # FFN2 psum -> value

    def vec_ap(dram, n):
        return bass.AP(tensor=dram, offset=0, ap=[[1, 128], [128, n // 128]])

    def wq_ap(wdram, ot):
        """Pre-tiled stationary block ot: [128, NT, 128]."""
        return bass.AP(tensor=wdram, offset=ot * 128 * C,
                       ap=[[C, 128], [128, NT], [1, 128]])

    def chunk_ap(dram, n, jp):
        """ct-pair chunk of a [C, n] tensor: [128, 2, n]."""
        return bass.AP(tensor=dram, offset=256 * jp * n,
                       ap=[[n, 128], [128 * n, 2], [1, n]])

    with ExitStack() as ctx:
        consts = ctx.enter_context(tc.tile_pool(name=f"con{it}", bufs=1))
        ones128 = consts.tile([128, 128], BF, tag="o128", name="o128")
        nc.vector.memset(ones128[:], 1.0)
        ones128r = consts.tile([128, 128], F32R, tag="o128r", name="o128r")
        nc.vector.memset(ones128r[:].bitcast(F32), 1.0)
        ones1 = consts.tile([1, 128], F32R, tag="o1", name="o1")
        nc.vector.memset(ones1[:].bitcast(F32), 1.0 / SV)
        eps_t = consts.tile([128, 1], F32, tag="eps", name="eps")
        nc.vector.memset(eps_t[:], 1e-5)
        expb = consts.tile([128, 1], F32, tag="expb", name="expb")
        nc.vector.memset(expb[:], math.log(SP))
        tri_sb = consts.tile([128, 64], F8, tag="tri", name="tri")

        def ldvec(key, n):
            s = consts.tile([128, n // 128], F32, tag=key, name=key)
            nc.gpsimd.dma_start(s[:], vec_ap(t[key], n))
            return s

        lv = {}

        def load_main_consts():
            for key, n in (("g1", C), ("b1", C), ("g2", C), ("b2", C),
                           ("g3", C), ("b3", C), ("csq2", C), ("bq2", C),
                           ("csf1", FF), ("bf1s", FF), ("bf2s", C)):
                lv[key] = ldvec(key, n)
            nc.gpsimd.dma_start(tri_sb[:], t["tri8"][:])

        pad_sb = {"pad1s": ldvec("pad1s", T), "pad2s": ldvec("pad2s", T)}

        wpool = ctx.enter_context(tc.tile_pool(name=f"wp{it}", bufs=4))
        foldp = ctx.enter_context(tc.tile_pool(name=f"fp{it}", bufs=2))
        statp = ctx.enter_context(tc.tile_pool(name=f"st{it}", bufs=1))

        # ---------- LN helpers ----------
        def ln_begin(sctx, tagp, lnps=None, lntag=None):
            sqp = sctx.enter_context(tc.tile_pool(name=f"sq{tagp}{it}", bufs=2))
            scr = sctx.enter_context(tc.tile_pool(name=f"ls{tagp}{it}", bufs=1))
            st = {"sqp": sqp, "scr": scr, "ps1": None, "ps2": None}
            if lnps is not None:
                ln_start_stats(st, lnps, lntag)
            return st

        def ln_start_stats(st, lnps, lntag):
            st["ps1"] = lnps.tile([128, TQ], F32, tag=lntag, name="ln")
            st["ps2"] = lnps.tile([128, TQ], F32, tag=lntag, name="ln")

        def ln_feed(st, y_tile, ct, f32=False):
            sq = st["sqp"].tile([128, TQ], BF, tag="sq", name="sq")
            with nc.allow_low_precision(reason="bf16 stats ok"):
                nc.vector.tensor_mul(sq[:], y_tile[:], y_tile[:])
            y_ap = y_tile[:].bitcast(F32R) if f32 else y_tile[:]
            ones = ones128r if f32 else ones128
            nc.tensor.matmul(st["ps1"][:], ones[:], y_ap,
                             start=(ct == 0), stop=(ct == NT - 1),
                             skip_group_check=True)
            nc.tensor.matmul(st["ps2"][:], ones128[:], sq[:],
                             start=(ct == 0), stop=(ct == NT - 1),
                             skip_group_check=True)

        def ln_finish(st, y_in, g, b, out_t, fold_c=None):
            """Stats; normalized+affine out_t (bf16).  With fold_c also
            produce (r2, un): r2 = rstd*fold_c, un = -m*rstd/SW."""
            scr = st["scr"]
            m = scr.tile([128, TQ], F32, tag="m", name="m")
            nc.vector.tensor_scalar_mul(m[:], st["ps1"][:], 1.0 / C)
            ms = scr.tile([128, TQ], F32, tag="v", name="v")
            nc.vector.tensor_scalar_mul(ms[:], st["ps2"][:], 1.0 / C)
            m2 = scr.tile([128, TQ], F32, tag="v2", name="v2")
            nc.vector.tensor_mul(m2[:], m[:], m[:])
            nc.vector.tensor_sub(ms[:], ms[:], m2[:])
            nc.scalar.activation(ms[:], ms[:], AF.Sqrt, bias=eps_t[:])
            rstd = scr.tile([128, TQ], F32, tag="r", name="r")
            nc.vector.reciprocal(rstd[:], ms[:])
            ret = None
            if fold_c is not None:
                un = statp.tile([128, TQ], F32, tag="un", name="un")
                nc.vector.tensor_mul(un[:], m[:], rstd[:])
                nc.vector.tensor_scalar(un[:], un[:], -1.0 / SW, None,
                                        op0=ALU.mult)
                r2 = statp.tile([128, TQ], F32, tag="r2", name="r2")
                nc.vector.tensor_scalar_mul(r2[:], rstd[:], fold_c)
                ret = (r2, un)
            for ct in range(NT):
                eng = nc.gpsimd if ct % 4 == 3 else nc.vector
                d = scr.tile([128, TQ], F32, tag="d", name="d", bufs=2)
                eng.tensor_sub(d[:], y_in[ct][:], m[:])
                eng.tensor_mul(d[:], d[:], rstd[:])
                nc.scalar.activation(out_t[ct][:], d[:], AF.Identity,
                                     bias=b[:, ct:ct + 1],
                                     scale=g[:, ct:ct + 1])
            return ret

        def ln_finish_chunked(st, y_in, g, b, out_t, nch=4):
            scr = st["scr"]
            m = scr.tile([128, TQ], F32, tag="m", name="m")
            ms = scr.tile([128, TQ], F32, tag="v", name="v")
            m2 = scr.tile([128, TQ], F32, tag="v2", name="v2")
            rstd = scr.tile([128, TQ], F32, tag="r", name="r")
            w = TQ // nch
            for cch in range(nch):
                sl = slice(cch * w, (cch + 1) * w)
                nc.vector.tensor_scalar_mul(m[:, sl], st["ps1"][:, sl], 1.0 / C)
                nc.vector.tensor_scalar_mul(ms[:, sl], st["ps2"][:, sl], 1.0 / C)
                nc.vector.tensor_mul(m2[:, sl], m[:, sl], m[:, sl])
                nc.vector.tensor_sub(ms[:, sl], ms[:, sl], m2[:, sl])
                nc.scalar.activation(ms[:, sl], ms[:, sl], AF.Sqrt,
                                     bias=eps_t[:])
                nc.vector.reciprocal(rstd[:, sl], ms[:, sl])
                for ct in range(NT):
                    eng = nc.gpsimd if ct % 2 == 1 else nc.vector
                    d = scr.tile([128, w], F32, tag="dc", name="dc", bufs=3)
                    eng.tensor_sub(d[:], y_in[ct][:, sl], m[:, sl])
                    eng.tensor_mul(d[:], d[:], rstd[:, sl])
                    nc.scalar.activation(out_t[ct][:, sl], d[:], AF.Identity,
                                         bias=b[:, ct:ct + 1],
                                         scale=g[:, ct:ct + 1])

        def fold_epilogue(ps, fold, cs, ot, out_ap, func, bias_sb, scale):
            """out = func(scale*(r2*ps + un*cs[ot]) + bias[ot])"""
            r2, un = fold
            eng = nc.vector if ot % 2 == 0 else nc.gpsimd
            ftile = foldp.tile([128, TQ], F32, tag="ft", name="ft")
            eng.tensor_mul(ftile[:], ps[:], r2[:])
            eng.scalar_tensor_tensor(
                out=ftile[:], in0=un[:], scalar=cs[:, ot:ot + 1], in1=ftile[:],
                op0=ALU.mult, op1=ALU.add)
            nc.scalar.activation(out_ap, ftile[:], func,
                                 bias=bias_sb[:, ot:ot + 1], scale=scale)

        # ---------- projection helpers ----------
        def qkv_proj(src8, w8_key, n, out_fn, pp):
            for ot in range(NT):
                wt = wpool.tile([128, NT, 128], F8, tag="w", name="w")
                nc.sync.dma_start(wt[:], wq_ap(t[w8_key], ot))
                for q0 in range(0, n, 512):
                    ps = pp.tile([128, 512], F32, tag="pp", name="pp")
                    for jp in range(4):
                        nc.tensor.matmul(
                            ps[:], wt[:, 2 * jp:2 * jp + 2, :],
                            src8[:, 2 * jp:2 * jp + 2, q0:q0 + 512],
                            start=(jp == 0), stop=(jp == 3), perf_mode=DR)
                    out_fn(ot, q0, ps)

        def v_proj(src8, wv_key, vtt, pad8key, padskey, pp):
            wvq = []
            for half in range(2):
                wvt = wpool.tile([128, NT, 512], F8, tag="wv", name="wv",
                                 bufs=2)
                nc.sync.dma_start(
                    wvt[:], bass.AP(tensor=t[wv_key], offset=512 * half,
                                    ap=[[C, 128], [128 * C, NT], [1, 512]]))
                wvq.append(wvt)
            for p in range(4):
                for s in range(2):
                    tt = 2 * p + s
                    nc.scalar.dma_start(
                        vtt[p][:, s, :, 64:65],
                        bass.AP(tensor=t[pad8key], offset=128 * tt,
                                ap=[[1, 128], [0, H], [0, 1]]))
                    for half in range(2):
                        ps = pp.tile([128, 512], F32, tag="pp", name="pp")
                        for jp in range(4):
                            nc.tensor.matmul(
                                ps[:],
                                src8[:, 2 * jp:2 * jp + 2,
                                     128 * tt:128 * (tt + 1)],
                                wvq[half][:, 2 * jp:2 * jp + 2, :],
                                start=(jp == 0), stop=(jp == 3), perf_mode=DR)
                        veng = nc.vector if half == 0 else nc.gpsimd
                        with nc.allow_low_precision(reason="fp8 V"):
                            veng.tensor_scalar_mul(
                                vtt[p][:, s, 8 * half:8 * (half + 1), 0:64],
                                ps[:].rearrange("p (h d) -> p h d", d=64),
                                pad_sb[padskey][:, tt:tt + 1])

        def attention(qTt, kTt, vtt, wo_key, resid, out_y, causal,
                      pps, avps, rbps, ops, lnst, post_co=None):
            with ExitStack() as atx:
                pvp = atx.enter_context(tc.tile_pool(name=f"pv{wo_key}{it}",
                                                     bufs=1))
                ppool = atx.enter_context(tc.tile_pool(name=f"pt{wo_key}{it}",
                                                       bufs=3))
                sbp = atx.enter_context(tc.tile_pool(name=f"sb{wo_key}{it}",
                                                     bufs=2))
                pv = [pvp.tile([128, TQ], BF, tag=f"pv{i}", name=f"pv{i}")
                      for i in range(NT)]

                def rb_tail(av, rinv, ct, off):
                    rb_ps = rbps.tile([128, 512], F32, tag="rb", name="rb")
                    nc.tensor.matmul(rb_ps[:], ones1[:], rinv[:],
                                     start=True, stop=True)
                    rb = sbp.tile([64, 512], F32, tag="rs", name="rs")
                    nc.vector.tensor_copy(rb[:], rb_ps[0:64, :])
                    with nc.allow_low_precision(reason="bf16 pv"):
                        nc.vector.tensor_mul(pv[ct][off:off + 64, :],
                                             av[0:64, :], rb[:])

                pend_rb = None
                for h_ in range(H):
                    ct, off = h_ // 2, (h_ % 2) * 64
                    av = avps.tile([65, 512], F32, tag="av", name="av")
                    pend = None

                    def emit_av(Pt, p):
                        c0p = 128 * p if causal else 0
                        nc.tensor.matmul(av[:, c0p:512],
                                         vtt[p][:, :, h_, 0:65],
                                         Pt[:, :, c0p:512],
                                         start=(p == 0), stop=(p == 3),
                                         perf_mode=DR)

                    for p in range(4):
                        Pt = ppool.tile([128, 2, TQ], F8, tag="P", name="P")
                        for s in range(2):
                            kt = 2 * p + s
                            c0 = 64 * kt if causal else 0
                            sp = pps.tile([128, 512], F32, tag="pp",
                                          name="pp")
                            nc.tensor.matmul(
                                sp[:, c0:512],
                                kTt[ct][off:off + 64,
                                        128 * kt:128 * (kt + 1)],
                                qTt[ct][off:off + 64, c0:512],
                                start=True, stop=True)
                            nc.scalar.activation(Pt[:, s, c0:512],
                                                 sp[:, c0:512], AF.Exp,
                                                 bias=expb[:], scale=0.125)
                            if causal:
                                if s == 1:
                                    nc.gpsimd.memset(
                                        Pt[:, 1, 128 * p:128 * p + 64], 0.0)
                                with nc.allow_low_precision(reason="fp8 P"):
                                    nc.gpsimd.tensor_mul(
                                        Pt[:, s, c0:c0 + 64],
                                        Pt[:, s, c0:c0 + 64], tri_sb[:])
                        if pend is not None:
                            emit_av(*pend)
                        pend = (Pt, p)
                    emit_av(*pend)
                    rinv = sbp.tile([1, 512], F32R, tag="ri", name="ri")
                    with nc.allow_low_precision(reason="f32r rinv"):
                        nc.vector.reciprocal(rinv[:], av[64:65, :])
                    if pend_rb is not None:
                        rb_tail(*pend_rb)
                    pend_rb = (av, rinv, ct, off)
                rb_tail(*pend_rb)
                ln_start_stats(lnst, avps, "av")
                for co in range(NT):
                    wt = wpool.tile([128, NT, 128], BF, tag="wo", name="wo",
                                    bufs=2)
                    nc.sync.dma_start(wt[:], wq_ap(t[wo_key], co))
                    ps = ops.tile([128, 512], F32, tag="o", name="o")
                    for cc in range(NT):
                        nc.tensor.matmul(ps[:], wt[:, cc, :], pv[cc][:],
                                         start=(cc == 0), stop=(cc == NT - 1))
                    with nc.allow_low_precision(reason="bf16 y"):
                        nc.vector.tensor_add(out_y[co][:], ps[:], resid(co))
                    ln_feed(lnst, out_y[co], co)
                    if post_co is not None:
                        post_co(co)

        # ================= main flow =================
        srcp = ctx.enter_context(tc.tile_pool(name=f"sp{it}", bufs=1))
        qpool = ctx.enter_context(tc.tile_pool(name=f"qp{it}", bufs=1))

        src_t = srcp.tile([128, NT, T], F8, tag="src", name="src")
        xl8_t = srcp.tile([128, NT, TQ], F8, tag="xl8", name="xl8")
        xlbf_t = srcp.tile([128, NT, TQ], BF, tag="xlbf", name="xlbf")
        enc_t = srcp.tile([128, NT, T], F8, tag="enc", name="enc")

        with ExitStack() as actx:
            qkv = actx.enter_context(tc.tile_pool(name=f"qkv{it}", bufs=1))
            pps = actx.enter_context(tc.tile_pool(
                name=f"ps{it}", bufs=3, space="PSUM"))
            avps = actx.enter_context(tc.tile_pool(
                name=f"as{it}", bufs=2, space="PSUM"))
            rbps = actx.enter_context(tc.tile_pool(
                name=f"rs{it}", bufs=1, space="PSUM"))
            ops = actx.enter_context(tc.tile_pool(
                name=f"os{it}", bufs=2, space="PSUM"))

            def new_kv(pfx):
                k = [qkv.tile([128, T], BF, tag=f"{pfx}k{i}", name=f"k{i}")
                     for i in range(NT)]
                v = [qkv.tile([128, 2, H, 65], F8, tag=f"{pfx}v{i}",
                              name=f"v{i}") for i in range(4)]
                return k, v

            qT = [qpool.tile([128, TQ], BF, tag=f"q{i}", name=f"q{i}")
                  for i in range(NT)]
            y18_t = qpool.tile([128, NT, TQ], F8, tag="y18", name="y18")
            qT2 = [qpool.tile([128, TQ], BF, tag=f"q2{i}", name=f"q2{i}")
                   for i in range(NT)]
            x1t = [qpool.tile([128, TQ], BF, tag=f"x1{i}", name=f"x1{i}")
                   for i in range(NT)]

            # ---- self-attention ----
            kTt, vtt = new_kv("s")
            with ExitStack() as sctx:
                for jp in range(4):
                    nc.sync.dma_start(xl8_t[:, 2 * jp:2 * jp + 2, :],
                                      chunk_ap(t["xl8"], TQ, jp))
                qkv_proj(xl8_t, "wq18", TQ,
                         lambda ot, q0, ps: nc.scalar.activation(
                             qT[ot][:], ps[:], AF.Identity, scale=cQK), pps)
                for jp in range(4):
                    nc.sync.dma_start(src_t[:, 2 * jp:2 * jp + 2, :],
                                      chunk_ap(t["xT8"], T, jp))

                def k_out(kTt_, use_act):
                    def f(ot, q0, ps):
                        if use_act:
                            nc.scalar.activation(kTt_[ot][:, q0:q0 + 512],
                                                 ps[:], AF.Identity,
                                                 scale=cQK)
                            return
                        eng = nc.vector if (ot + q0 // 512) % 2 == 0 \
                            else nc.gpsimd
                        with nc.allow_low_precision(reason="bf16 K"):
                            eng.tensor_scalar_mul(
                                kTt_[ot][:, q0:q0 + 512], ps[:], cQK)
                    return f

                qkv_proj(src_t, "wk18", T, k_out(kTt, True), pps)
                load_main_consts()
                v_proj(src_t, "wv18", vtt, "pad18", "pad1s", pps)
                for jp in range(4):
                    nc.gpsimd.dma_start(xlbf_t[:, 2 * jp:2 * jp + 2, :],
                                        chunk_ap(t["xl_bf"], TQ, jp))
                    nc.gpsimd.dma_start(enc_t[:, 2 * jp:2 * jp + 2, :],
                                        chunk_ap(t["encT8"], T, jp))
                lnst1 = ln_begin(sctx, "s")

                def y18_quant(co):
                    nc.scalar.activation(y18_t[:, co, :], qT[co][:],
                                         AF.Identity, scale=SY1)

                attention(qT, kTt, vtt, "wo1bf",
                          lambda co: xlbf_t[:, co, :], qT, True,
                          pps, avps, rbps, ops, lnst1, post_co=y18_quant)
                # K2/V2 fill the LN1-chain bubble on the PE
                kTt2, vtt2 = new_kv("c")
                qkv_proj(enc_t, "wk28", T, k_out(kTt2, False), pps)
                fold1 = ln_finish(lnst1, qT, lv["g1"], lv["b1"], x1t,
                                  fold_c=cQ2)
                v_proj(enc_t, "wv28", vtt2, "pad28", "pad2s", pps)
                # Q2 with LN1 folded through Wq2
                for ot in range(NT):
                    wt = wpool.tile([128, NT, 128], F8, tag="w", name="w")
                    nc.sync.dma_start(wt[:], wq_ap(t["wq28"], ot))
                    ps = pps.tile([128, 512], F32, tag="pp", name="pp")
                    for jp in range(4):
                        nc.tensor.matmul(
                            ps[:], wt[:, 2 * jp:2 * jp + 2, :],
                            y18_t[:, 2 * jp:2 * jp + 2, :],
                            start=(jp == 0), stop=(jp == 3), perf_mode=DR)
                    fold_epilogue(ps, fold1, lv["csq2"], ot, qT2[ot][:],
                                  AF.Identity, lv["bq2"], 1.0)

            # ---- cross-attention ----
            with ExitStack() as cctx:
                lnst2 = ln_begin(cctx, "c")
                y2a_t = qpool.tile([128, NT, TQ], F8, tag="y2a", name="y2a")
                y2b_t = qpool.tile([128, NT, TQ], F8, tag="y2b", name="y2b")

                def y2_quant(co):
                    nc.scalar.activation(y2a_t[:, co, :], qT2[co][:],
                                         AF.Identity, scale=SY2)
                    with nc.allow_low_precision(reason="fp8 y2b"):
                        nc.gpsimd.scalar_tensor_tensor(
                            out=y2b_t[:, co, :], in0=qT2[co][:], scalar=SY2,
                            in1=y2a_t[:, co, :], op0=ALU.mult,
                            op1=ALU.subtract)

                attention(qT2, kTt2, vtt2, "wo2bf",
                          lambda co: x1t[co][:], qT2, False,
                          pps, avps, rbps, ops, lnst2, post_co=y2_quant)
                fold2 = ln_finish(lnst2, qT2, lv["g2"], lv["b2"], x1t,
                                  fold_c=cF1)

        # ---- FFN + AddNorm ----  (x1t now holds x2 = LN2 out)
        with ExitStack() as fctx:
            y3p = fctx.enter_context(tc.tile_pool(name=f"y3{it}", bufs=1))
            y3 = [y3p.tile([128, TQ], F32, tag=f"z{i}", name=f"z{i}")
                  for i in range(NT)]
            h8_t = y3p.tile([128, FT, TQ], F8, tag="h8", name="h8")
            lnps3 = fctx.enter_context(tc.tile_pool(
                name=f"l3{it}", bufs=2, space="PSUM"))
            lnst3 = ln_begin(fctx, "f", lnps3, "ln")
            with ExitStack() as mctx:
                w1p = mctx.enter_context(tc.tile_pool(name=f"w1{it}", bufs=4))
                w2p = mctx.enter_context(tc.tile_pool(name=f"w2{it}", bufs=3))
                pp1 = mctx.enter_context(tc.tile_pool(
                    name=f"f1{it}", bufs=3, space="PSUM"))
                pp2 = mctx.enter_context(tc.tile_pool(
                    name=f"f2{it}", bufs=2, space="PSUM"))
                w1tiles = []
                for fg in range(FT):
                    w1t = w1p.tile([128, 2, NT, 128], F8, tag="w1", name="w1")
                    nc.sync.dma_start(
                        w1t[:],
                        bass.AP(tensor=t["wf1ab"], offset=fg * 128 * 2 * C,
                                ap=[[2 * C, 128], [C, 2], [128, NT],
                                    [1, 128]]))
                    w1tiles.append(w1t)
                w2tiles = []
                for fb in range(2):
                    for co in range(NT):
                        w2t = w2p.tile([128, 2, 16, 128], F8, tag="w2",
                                       name="w2")
                        nc.sync.dma_start(
                            w2t[:],
                            bass.AP(tensor=t["wf2ab"],
                                    offset=co * 128 * 8192 + fb * 2048,
                                    ap=[[8192, 128], [4096, 2], [128, 16],
                                        [1, 128]]))
                        w2tiles.append(w2t)
                for fg in range(FT):
                    w1t = w1tiles[fg]
                    ps = pp1.tile([128, 512], F32, tag="p1", name="p1")
                    for term, (ab, ysrc) in enumerate(
                            ((0, y2a_t), (1, y2a_t), (0, y2b_t))):
                        for jp in range(4):
                            nc.tensor.matmul(
                                ps[:], w1t[:, ab, 2 * jp:2 * jp + 2, :],
                                ysrc[:, 2 * jp:2 * jp + 2, :],
                                start=(term == 0 and jp == 0),
                                stop=(term == 2 and jp == 3), perf_mode=DR)
                    fold_epilogue(ps, fold2, lv["csf1"], fg,
                                  h8_t[:, fg, :], AF.Relu, lv["bf1s"], SH)
                for fb in range(2):
                    for co in range(NT):
                        w2t = w2tiles[fb * NT + co]
                        ps = pp2.tile([128, 512], F32, tag="p2", name="p2")
                        for ab in range(2):
                            for jp in range(8):
                                nc.tensor.matmul(
                                    ps[:], w2t[:, ab, 2 * jp:2 * jp + 2, :],
                                    h8_t[:, 16 * fb + 2 * jp:
                                         16 * fb + 2 * jp + 2, :],
                                    start=(ab == 0 and jp == 0),
                                    stop=(ab == 1 and jp == 7), perf_mode=DR)
                        if fb == 0:
                            nc.vector.tensor_scalar_mul(y3[co][:], ps[:], cF2)
                        else:
                            nc.vector.scalar_tensor_tensor(
                                out=y3[co][:], in0=ps[:], scalar=cF2,
                                in1=y3[co][:], op0=ALU.mult, op1=ALU.add)
                            nc.vector.scalar_tensor_tensor(
                                out=y3[co][:], in0=y3[co][:],
                                scalar=lv["bf2s"][:, co:co + 1],
                                in1=x1t[co][:], op0=ALU.add, op1=ALU.add)
                            ln_feed(lnst3, y3[co], co, f32=True)
            out_f = [y3p.tile([128, TQ], F32, tag=f"of{i}", name=f"of{i}")
                     for i in range(NT)]
            ln_finish_chunked(lnst3, y3, lv["g3"], lv["b3"], out_f, nch=2)
            for cch in range(2):
                sl = slice(cch * (TQ // 2), (cch + 1) * (TQ // 2))
                for co in range(NT):
                    eng = nc.sync if co % 2 == 0 else nc.scalar
                    eng.dma_start(
                        t["outT"][128 * co:128 * (co + 1), sl],
                        out_f[co][:, sl])


def _q8(x, s):
    import ml_dtypes
    y = np.asarray(np.asarray(x, np.float32) * np.float32(s),
                   ml_dtypes.float8_e4m3)
    assert np.isfinite(y.astype(np.float32)).all()
    return y


def _tile_w(w):
    """[cin, cout] -> [cout//128, 128, cin]: block[ot][p][128*ct+c] =
    w[128*ct+c, 128*ot+... block[ot, p, k] = w[k, 128*ot+p] is WRONG;
    stationary wants block[ot, p, k] such that tile[p, ct, c] =
    w[128*ct+p, 128*ot+c], i.e. block[ot, p, ct*128+c]."""
    cin, cout = w.shape
    return np.ascontiguousarray(
        w.reshape(cin // 128, 128, cout // 128, 128)
        .transpose(2, 1, 0, 3).reshape(cout // 128, 128, cin))


def _shard(inputs):
    import ml_dtypes
    F8NP = ml_dtypes.float8_e4m3
    BFNP = ml_dtypes.bfloat16
    x = np.asarray(inputs["x"], dtype=np.float32)
    enc = np.asarray(inputs["enc_out"], dtype=np.float32)
    tpad = np.asarray(inputs["tgt_pad_mask"]).astype(np.float32)
    spad = np.asarray(inputs["src_pad_mask"]).astype(np.float32)
    ws = {k: np.asarray(inputs[k], dtype=np.float32)
          for k in ("Wq1", "Wk1", "Wv1", "Wo1", "Wq2", "Wk2", "Wv2", "Wo2",
                    "Wf1", "Wf2")}
    lnv = {k: np.asarray(inputs[k], dtype=np.float32)
           for k in ("ln1_g", "ln1_b", "ln2_g", "ln2_b", "ln3_g", "ln3_b",
                     "bf1", "bf2")}
    cV = SV / (SX * SW)

    w8 = {}
    for src, dst in (("Wq1", "wq18"), ("Wk1", "wk18"), ("Wk2", "wk28")):
        w8[dst] = _tile_w(_q8(ws[src], SW))
    w8["wv18"] = np.ascontiguousarray(_q8(ws["Wv1"], SW))
    w8["wv28"] = np.ascontiguousarray(_q8(ws["Wv2"], SW))
    wq2f = lnv["ln1_g"][:, None] * ws["Wq2"]
    wq2q = _q8(wq2f, SW)
    w8["wq28"] = _tile_w(wq2q)
    csq2 = np.ascontiguousarray(wq2q.astype(np.float32).sum(axis=0))
    bq2 = np.ascontiguousarray(lnv["ln1_b"] @ ws["Wq2"])
    wo1bf = np.ascontiguousarray(_tile_w(ws["Wo1"]).astype(BFNP))
    wo2bf = np.ascontiguousarray(_tile_w(ws["Wo2"]).astype(BFNP))
    wf1f = lnv["ln2_g"][:, None] * ws["Wf1"]
    w1a = _q8(wf1f, SW)
    w1b = _q8(wf1f * SW - w1a.astype(np.float32), 1.0)
    csf1 = np.ascontiguousarray(
        (w1a.astype(np.float32) + w1b.astype(np.float32)).sum(axis=0))
    wf1ab = np.ascontiguousarray(np.concatenate(
        [_tile_w(w1a), _tile_w(w1b)], axis=2))          # [FT,128,2C]
    bf1s = np.ascontiguousarray(
        SH * (lnv["bf1"] + lnv["ln2_b"] @ ws["Wf1"]))
    w2a = _q8(ws["Wf2"], SW)
    w2b = _q8(ws["Wf2"] * SW - w2a.astype(np.float32), 1.0)
    ta, tb = _tile_w(w2a), _tile_w(w2b)                  # [8,128,4096]
    wf2ab = np.ascontiguousarray(
        np.stack([ta.reshape(NT, 128, 2, 2048),
                  tb.reshape(NT, 128, 2, 2048)], axis=2)
        .reshape(NT, 128, 2 * 2 * 2048))
    in_maps = []
    for b in range(B):
        xT8 = np.ascontiguousarray(_q8(x[b].T, SX))
        eT8 = np.ascontiguousarray(_q8(enc[b].T, SX))
        p1s = np.ascontiguousarray((1.0 - tpad[b]) * cV)
        p2s = np.ascontiguousarray((1.0 - spad[b]) * cV)
        p18 = np.ascontiguousarray((1.0 - tpad[b]).astype(F8NP))
        p28 = np.ascontiguousarray((1.0 - spad[b]).astype(F8NP))
        for h in range(2):
            xl = x[b, h::2, :].T
            trih = (np.arange(128)[:, None] <= 2 * np.arange(64)[None, :] + h
                    ).astype(F8NP)
            in_maps.append({
                "xT8": xT8, "xl8": np.ascontiguousarray(_q8(xl, SX)),
                "encT8": eT8,
                "xl_bf": np.ascontiguousarray(xl.astype(BFNP)),
                **w8, "wo1bf": wo1bf, "wo2bf": wo2bf,
                "wf1ab": wf1ab, "wf2ab": wf2ab,
                "csq2": csq2, "bq2": bq2, "csf1": csf1,
                "bf1s": bf1s, "bf2s": lnv["bf2"],
                "tri8": np.ascontiguousarray(trih),
                "pad18": p18, "pad28": p28,
                "pad1s": p1s, "pad2s": p2s,
                "g1": lnv["ln1_g"], "b1": lnv["ln1_b"],
                "g2": lnv["ln2_g"], "b2": lnv["ln2_b"],
                "g3": lnv["ln3_g"], "b3": lnv["ln3_b"],
            })
    return in_maps


def _get_nc(repeat=1):
    if repeat not in _CACHE:
        _CACHE[repeat] = _build(repeat)
    return _CACHE[repeat]


def kernel(**inputs):
    from concourse.bass_utils import run_bass_kernel_spmd
    nc = _get_nc()
    in_maps = _shard(inputs)
    res = run_bass_kernel_spmd(nc, in_maps, core_ids=list(range(8)))
    out = np.empty((B, T, C), np.float32)
    for core in range(8):
        b, h = core // 2, core % 2
        out[b, h::2, :] = res.results[core]["outT"].T
    return out
